# revision 1
# baseline (speedup 1.0000x reference)
"""Trainium2 Bass kernel for nn_AttentionBlock (GroupNorm + per-position
head-axis attention + proj + residual).

Sharding: data-parallel over batch B=16 -> 2 batches per core x 8 cores.
Each core runs an identical program on its x-shard [2, 512, 4096] plus
replicated (host-preprocessed) weights, and writes its out-shard.

Per-core pipeline:
  1. GroupNorm(32): bn_stats per partition over N; cross-partition group
     aggregation via two tiny SBUF->SBUF DMA gathers (DMA crosses
     partitions); normalize on ACT with per-partition scale/bias.
  2. QKV: out[n, o] via PE with h-block stationary -> QKV arrives N-major.
     h is normalized into two half-batch column groups (low half
     double-buffered) so consecutive batches overlap.
  3. Attention (N-major, per 128-position block): logits/AV as broadcast
     elementwise multiplies (bf16 unit-stride so the DVE 2x mode applies;
     the V weight columns are host-permuted to [d*8+g] for this), with the
     d- and g-reductions done as in-place halving add-trees (adds get 2x
     mode; InstTensorReduce would run 1x). Softmax skips max-subtraction
     (logits are O(1) by construction); Exp on ACT with the 1/8 scale
     folded in. The logits multiply runs on GPSIMD for 2/3 of blocks.
  4. O transposed back to C-major via PE transpose; proj matmul on PE
     consumes a 3-deep ring of per-chunk OT tiles; residual-add fused into
     the PSUM->SBUF eviction on DVE (x re-DMA'd per chunk); DMA out.

Host-side preprocessing: weight transposes + bf16 casts + V-column permute.
If qkv_b is nonzero the kernel emits bias adds (specialized at trace; the
benchmark uses zero biases).

_cap_sync_waits: this walrus build accepts only ONE sync wait per compute
instruction; Tile emits more. The pass hoists excess waits onto same-engine
InstNoOps inserted immediately before the offender.
"""

import os

import numpy as np
import ml_dtypes

import concourse.bass as bass
import concourse.mybir as mybir
import concourse.tile as tile
from concourse.bass_utils import run_bass_kernel_spmd

F32 = mybir.dt.float32
BF16 = mybir.dt.bfloat16

B, C, HH, WW = 16, 512, 64, 64
N = HH * WW            # 4096
NB = 2                 # batches per core
NCORES = 8
NH, HD = 8, 64         # heads, head dim
GROUPS = 32
GSIZE = C // GROUPS    # 16 channels per group
EPS = 1e-5
CT = C // 128          # 4 channel tiles
OT3 = 3 * C // 512     # 3 o-chunks of 512 in qkv
NBLK = N // 128        # 32 position blocks per batch

AX = mybir.AxisListType
ALU = mybir.AluOpType
ACTF = mybir.ActivationFunctionType


def _bc(t, dims):
    """AP over tile/AP `t` with explicit free [step,count] dims (elem units)."""
    return bass.AP(tensor=t.tensor, offset=t.offset,
                   ap=[list(t.ap[0])] + [list(d) for d in dims])


def _cap_sync_waits(nc):
    """Walrus (this neuronxcc) allows at most 2 sync waits per compute
    instruction and is stricter still for some DMA structs. Tile can emit
    more. Hoist the excess onto a same-engine InstNoOp inserted immediately
    before the offender — the waits still complete before it executes."""
    import bass_rust
    n = 0
    for f in nc.m.functions:
        for blk in f.blocks:
            il = blk.instructions
            i = 0
            while i < len(il):
                ins = il[i]
                si = getattr(ins, "sync_info", None)
                if si is not None and si.on_wait and len(si.on_wait) > 1:
                    waits = list(si.on_wait)
                    for w in waits[:-1]:
                        nop = mybir.InstNoOp(name=f"W-abs-{n}", ins=[], outs=[])
                        n += 1
                        nop.engine = ins.engine
                        nop.sync_info = bass_rust.SyncInfo(on_wait=[w],
                                                           on_update=[])
                        il.insert(i, nop)
                        i += 1
                    si.on_wait = waits[-1:]
                i += 1
    return n


def build_kernel(nb=NB, nblk=NBLK, qk_bias=False, debug=False):
    n = nblk * 128
    cs = min(512, n)       # proj/residual n-chunk
    nch = n // cs
    nc = bass.Bass()
    dbg = {}
    if debug:
        dbg["h"] = nc.dram_tensor("dbg_h", [C, n], F32, kind="ExternalOutput")
        dbg["qkv"] = nc.dram_tensor("dbg_qkv", [128, 3 * C], F32, kind="ExternalOutput")
        dbg["s"] = nc.dram_tensor("dbg_s", [128, NH * NH], F32, kind="ExternalOutput")
        dbg["a"] = nc.dram_tensor("dbg_a", [128, NH * NH], F32, kind="ExternalOutput")
        dbg["o"] = nc.dram_tensor("dbg_o", [128, C], F32, kind="ExternalOutput")
        dbg["otsb"] = nc.dram_tensor("dbg_otsb", [C, n], F32, kind="ExternalOutput")

    x_d = nc.dram_tensor("x", [nb, C, n], F32, kind="ExternalInput")
    wqkvT_d = nc.dram_tensor("wqkvT", [C, 3 * C], BF16, kind="ExternalInput")
    pwT_d = nc.dram_tensor("pwT", [C, C], BF16, kind="ExternalInput")
    normw_d = nc.dram_tensor("normw", [C], F32, kind="ExternalInput")
    normb_d = nc.dram_tensor("normb", [C], F32, kind="ExternalInput")
    qkvb_d = nc.dram_tensor("qkvb", [3 * C], F32, kind="ExternalInput")
    pbeff_d = nc.dram_tensor("pbeff", [C], F32, kind="ExternalInput")
    ident_d = nc.dram_tensor("ident", [128, 128], BF16, kind="ExternalInput")
    out_d = nc.dram_tensor("out", [nb, C, n], F32, kind="ExternalOutput")

    with tile.TileContext(nc) as tc:
        with (
            tc.tile_pool(name="consts", bufs=1) as consts,
            tc.tile_pool(name="xpool", bufs=1) as xpool,
            tc.tile_pool(name="hlo", bufs=2) as hlo,
            tc.tile_pool(name="hhi", bufs=1) as hhi,
            tc.tile_pool(name="otr", bufs=3) as otr,
            tc.tile_pool(name="stats", bufs=2) as stats,
            tc.tile_pool(name="scb", bufs=4) as scb,
            tc.tile_pool(name="qkvsb", bufs=3) as qkvsb,
            tc.tile_pool(name="upool", bufs=4) as upool,
            tc.tile_pool(name="spool", bufs=4) as spool,
            tc.tile_pool(name="opool", bufs=4) as opool,
            tc.tile_pool(name="outsb", bufs=2) as outsb,
            tc.tile_pool(name="pqkv", bufs=4, space="PSUM") as pqkv,   # 6 banks
            tc.tile_pool(name="pmm", bufs=2, space="PSUM") as pmm,     # 2 banks
        ):
            # ---- constants / weights in SBUF ----
            wqkvT = []
            for c in range(CT):
                t = consts.tile([128, 3 * C], BF16, tag=f"wq{c}")
                nc.sync.dma_start(out=t, in_=wqkvT_d[c * 128:(c + 1) * 128, :])
                wqkvT.append(t)
            pwT = []
            for o in range(CT):
                t = consts.tile([128, C], BF16, tag=f"pw{o}")
                nc.sync.dma_start(out=t, in_=pwT_d[o * 128:(o + 1) * 128, :])
                pwT.append(t)
            ident = consts.tile([128, 128], BF16, tag="ident")
            nc.sync.dma_start(out=ident, in_=ident_d[:, :])
            nwt, nbt, pbt = [], [], []
            for c in range(CT):
                sl = slice(c * 128, (c + 1) * 128)
                t1 = consts.tile([128, 1], F32, tag=f"nw{c}")
                nc.sync.dma_start(out=t1, in_=normw_d[sl].rearrange("(p u) -> p u", u=1))
                nwt.append(t1)
                t2 = consts.tile([128, 1], F32, tag=f"nb{c}")
                nc.sync.dma_start(out=t2, in_=normb_d[sl].rearrange("(p u) -> p u", u=1))
                nbt.append(t2)
                t3 = consts.tile([128, 1], F32, tag=f"pb{c}")
                nc.sync.dma_start(out=t3, in_=pbeff_d[sl].rearrange("(p u) -> p u", u=1))
                pbt.append(t3)
            epst = consts.tile([1, 1], F32, tag="eps")
            nc.vector.memset(epst, 256.0 * EPS)
            qkbias = None
            if qk_bias:
                qkbias = consts.tile([128, 3 * C], F32, tag="qkb")
                nc.sync.dma_start(
                    out=qkbias,
                    in_=bass.AP(tensor=qkvb_d.ap().tensor, offset=0,
                                ap=[[0, 128], [1, 3 * C]]))

            hb = max(1, nblk // 2)          # blocks per half
            nh2 = hb * 128

            def emit_head(b):
                    # ---------- load x, GroupNorm ----------
                    xt, scale_t, bias_t = [], [], []  # noqa
                    for c in range(CT):
                        t = xpool.tile([128, n], F32, tag=f"x{c}")
                        nc.sync.dma_start(out=t, in_=x_d[b, c * 128:(c + 1) * 128, :])
                        xt.append(t)
                    for c in range(CT):
                        nsub = max(1, n // 512)
                        sd = nc.vector.BN_STATS_DIM
                        st = stats.tile([128, nsub, sd], F32, tag="bnst")
                        xv = xt[c].rearrange("p (s f) -> p s f", s=nsub)
                        for s in range(nsub):
                            nc.vector.bn_stats(out=st[:, s, :], in_=xv[:, s, :])
                        mv = stats.tile([128, nc.vector.BN_AGGR_DIM], F32, tag="bnmv")
                        nc.vector.bn_aggr(out=mv, in_=st)
                        # st2: col0 = mean, col1 = E[x^2] = var + mean^2
                        st2 = stats.tile([128, 2], F32, tag="st2")
                        nc.vector.tensor_copy(out=st2[:, 0:1], in_=mv[:, 0:1])
                        nc.vector.scalar_tensor_tensor(
                            out=st2[:, 1:2], in0=mv[:, 0:1], scalar=mv[:, 0:1],
                            in1=mv[:, 1:2], op0=ALU.mult, op1=ALU.add)
                        # gather all 128 partitions' stats onto one partition (DMA
                        # crosses partitions; avoids PE for the group aggregation)
                        stT = stats.tile([1, 256], F32, tag="stT")
                        nc.gpsimd.dma_start(out=stT, in_=st2)
                        # per-group sums over the 16 channels: [1, 8, 2]
                        gsum = stats.tile([1, 16], F32, tag="gsum")
                        nc.vector.tensor_reduce(
                            out=gsum.rearrange("p (g c) -> p g c", g=8),
                            in_=_bc(stT, [(32, 8), (1, 2), (2, 16)]),
                            axis=AX.X, op=ALU.add)
                        gm = _bc(gsum, [(2, 8)])           # sum of means      [1,8]
                        ge = bass.AP(tensor=gsum.tensor, offset=gsum.offset + 1,
                                     ap=[list(gsum.ap[0])] + [[2, 8]])  # sum E[x^2]
                        m2 = stats.tile([1, 8], F32, tag="m2")
                        nc.vector.tensor_mul(m2, gm, gm)
                        # 256*var = 16*sum_ex2 - (sum_mean)^2
                        v256 = stats.tile([1, 8], F32, tag="v256")
                        nc.vector.scalar_tensor_tensor(
                            out=v256, in0=ge, scalar=16.0, in1=m2,
                            op0=ALU.mult, op1=ALU.subtract)
                        sg = stats.tile([1, 8], F32, tag="sg")
                        nc.scalar.activation(out=sg, in_=v256, func=ACTF.Sqrt,
                                             scale=1.0, bias=epst)   # sqrt(256(var+eps))
                        rg = stats.tile([1, 8], F32, tag="rg")
                        nc.vector.reciprocal(out=rg, in_=sg)          # rstd/16
                        # broadcast to 128 channel slots, interleaved (mean, rstd)
                        sb2 = stats.tile([1, 256], F32, tag="sb2")
                        nc.vector.tensor_scalar(
                            out=_bc(sb2, [(32, 8), (2, 16)]),
                            in0=_bc(gsum, [(2, 8), (0, 16)]), scalar1=1.0 / 16.0,
                            scalar2=None, op0=ALU.mult)
                        nc.vector.tensor_scalar(
                            out=bass.AP(tensor=sb2.tensor, offset=sb2.offset + 1,
                                        ap=[list(sb2.ap[0])] + [[32, 8], [2, 16]]),
                            in0=_bc(rg, [(1, 8), (0, 16)]), scalar1=16.0,
                            scalar2=None, op0=ALU.mult)
                        pb2 = stats.tile([128, 2], F32, tag="pb2")
                        nc.gpsimd.dma_start(out=pb2, in_=sb2)
                        sc = scb.tile([128, 1], F32, tag="sc", name="sc")
                        bi = scb.tile([128, 1], F32, tag="bi", name="bi")
                        tmp = stats.tile([128, 1], F32, tag="tmp")
                        nc.vector.tensor_mul(sc, pb2[:, 1:2], nwt[c])
                        nc.vector.tensor_mul(tmp, pb2[:, 0:1], sc)
                        nc.vector.tensor_sub(bi, nbt[c], tmp)
                        scale_t.append(sc)
                        bias_t.append(bi)
                    return xt, scale_t, bias_t

            def emit_norm(b, half, xt, scale_t, bias_t):
                hs = slice(half * nh2, min(n, (half + 1) * nh2))
                out = []
                for c in range(CT):
                    hp = hlo if half == 0 else hhi
                    t = hp.tile([128, nh2], BF16, tag=f"h{half}_{c}",
                                name=f"h{half}_{c}")
                    nc.scalar.activation(out=t, in_=xt[c][:, hs],
                                         func=ACTF.Identity,
                                         bias=bias_t[c], scale=scale_t[c])
                    out.append(t)
                    if debug and b == 0:
                        hf = stats.tile([128, nh2], F32, tag="dbgh", name="hf")
                        nc.vector.tensor_copy(out=hf, in_=t)
                        nc.sync.dma_start(
                            out=dbg["h"][c * 128:(c + 1) * 128, hs], in_=hf)
                return out

            for b in range(nb):
                xt, scale_t, bias_t = emit_head(b)
                ht = [emit_norm(b, 0, xt, scale_t, bias_t), None]
                if nblk > 1:
                    ht[1] = emit_norm(b, 1, xt, scale_t, bias_t)
                else:
                    ht[1] = ht[0]
                # ---------- per 128-position block ----------
                bpc = cs // 128
                otc = None
                for blk in range(nblk):
                    ns = slice(blk * 128, (blk + 1) * 128)
                    if blk % bpc == 0:
                        otc = [otr.tile([128, cs], BF16, tag=f"otr{ob}",
                                        name=f"otr{ob}") for ob in range(CT)]
                    half = min(blk // hb, 1)
                    hslice = slice(blk * 128 - half * nh2, (blk + 1) * 128 - half * nh2)
                    pqc = [pqkv.tile([128, 512], F32, tag="pq", name=f"pq{oc}")
                           for oc in range(OT3)]
                    for c in range(CT):
                        lhsT = ht[half][c][:, hslice]
                        for oc in range(OT3):
                            nc.tensor.matmul(
                                pqc[oc], lhsT,
                                wqkvT[c][:, oc * 512:(oc + 1) * 512],
                                start=(c == 0), stop=(c == CT - 1))
                    qkv = qkvsb.tile([128, 3 * C], BF16, tag="qkv")
                    for oc in range(OT3):
                        if qkbias is not None:
                            nc.vector.tensor_add(
                                out=qkv[:, oc * 512:(oc + 1) * 512], in0=pqc[oc],
                                in1=qkbias[:, oc * 512:(oc + 1) * 512])
                        else:
                            nc.scalar.copy(
                                out=qkv[:, oc * 512:(oc + 1) * 512], in_=pqc[oc])

                    q = qkv[:, 0:512]
                    k = qkv[:, 512:1024]
                    v = qkv[:, 1024:1536]

                    # logits: U1[(h,g,d)] = q[h,d] * k[g,d]
                    u1 = upool.tile([128, NH * NH * HD], BF16, tag="u")
                    u1eng = nc.gpsimd if True else nc.vector
                    u1eng.tensor_tensor(
                        out=u1.rearrange("p (h g d) -> p h g d", h=NH, g=NH),
                        in0=_bc(q, [(HD, NH), (0, NH), (1, HD)]),
                        in1=_bc(k, [(0, NH), (HD, NH), (1, HD)]),
                        op=ALU.mult)
                    u1v = u1.rearrange("p (a d) -> p a d", d=HD)
                    w = HD
                    while w > 2:
                        nc.vector.tensor_tensor(
                            out=u1v[:, :, 0:w // 2], in0=u1v[:, :, 0:w // 2],
                            in1=u1v[:, :, w // 2:w], op=ALU.add)
                        w //= 2
                    s_l = spool.tile([128, NH * NH], F32, tag="s")
                    nc.vector.tensor_tensor(
                        out=s_l.rearrange("p (a u) -> p a u", u=1),
                        in0=u1v[:, :, 0:1], in1=u1v[:, :, 1:2], op=ALU.add)
                    # softmax over g: E = exp(S/8); logits bounded so no max-sub
                    e_l = spool.tile([128, NH * NH], BF16, tag="e")
                    nc.scalar.activation(out=e_l, in_=s_l, func=ACTF.Exp,
                                         scale=0.125)
                    d_l = spool.tile([128, NH], F32, tag="d")
                    nc.vector.tensor_reduce(
                        out=d_l, in_=e_l.rearrange("p (h g) -> p h g", g=NH),
                        axis=AX.X, op=ALU.add)
                    r_l = spool.tile([128, NH], F32, tag="r")
                    nc.vector.reciprocal(out=r_l, in_=d_l)
                    a_l = spool.tile([128, NH * NH], BF16, tag="a")
                    nc.vector.tensor_tensor(
                        out=a_l.rearrange("p (h g) -> p h g", g=NH),
                        in0=e_l.rearrange("p (h g) -> p h g", g=NH),
                        in1=_bc(r_l, [(1, NH), (0, NH)]),
                        op=ALU.mult)
                    # AV: U2[(h,d,g)] = A[h,g] * V'[d,g]; O = sum_g
                    # (V columns host-permuted to [d*8+g] so both reads are
                    # unit-stride innermost -> DVE 2x mode)
                    u2 = upool.tile([128, NH * HD * NH], BF16, tag="u")
                    nc.vector.tensor_tensor(
                        out=u2.rearrange("p (h d g) -> p h d g", h=NH, d=HD),
                        in0=_bc(a_l, [(NH, NH), (0, HD), (1, NH)]),
                        in1=_bc(v, [(0, NH), (NH, HD), (1, NH)]),
                        op=ALU.mult)
                    # O = sum_g via in-place halving adds (2x-mode eligible)
                    uv = u2.rearrange("p (a g) -> p a g", g=NH)
                    w = NH
                    while w > 2:
                        nc.vector.tensor_tensor(
                            out=uv[:, :, 0:w // 2], in0=uv[:, :, 0:w // 2],
                            in1=uv[:, :, w // 2:w], op=ALU.add)
                        w //= 2
                    o_l = opool.tile([128, C], BF16, tag="o")
                    nc.vector.tensor_tensor(
                        out=o_l.rearrange("p (a u) -> p a u", u=1),
                        in0=uv[:, :, 0:1], in1=uv[:, :, 1:2], op=ALU.add)
                    if debug and b == 0 and blk == 0:
                        for nm, src in (("qkv", qkv), ("s", s_l), ("a", a_l), ("o", o_l)):
                            ff = stats.tile(list(src.shape), F32, tag=f"dbg{nm}",
                                            name=f"f{nm}")
                            nc.vector.tensor_copy(out=ff, in_=src)
                            nc.sync.dma_start(out=dbg[nm][:, :], in_=ff)
                    # transpose O back to C-major
                    pt = pmm.tile([128, 512], BF16, tag="pt")
                    for ob in range(CT):
                        nc.tensor.transpose(pt[:, ob * 128:(ob + 1) * 128],
                                            o_l[:, ob * 128:(ob + 1) * 128], ident)
                    for ob in range(CT):
                        nc.scalar.copy(
                            out=otc[ob][:, (blk % bpc) * 128:(blk % bpc + 1) * 128],
                            in_=pt[:, ob * 128:(ob + 1) * 128])

                    # proj + residual for chunk j as soon as its 4 blocks of
                    # OT columns exist (x re-DMA'd per chunk; x tiles free
                    # after norm)
                    if (blk + 1) % bpc == 0:
                        j = blk // bpc
                        ncs = slice(j * cs, (j + 1) * cs)
                        for c in range(CT):
                            xr = outsb.tile([128, cs], F32, tag="xr", name="xr")
                            nc.sync.dma_start(out=xr,
                                              in_=x_d[b, c * 128:(c + 1) * 128, ncs])
                            py = pmm.tile([128, cs], F32, tag="py", name="py")
                            for ob in range(CT):
                                nc.tensor.matmul(py,
                                                 pwT[ob][:, c * 128:(c + 1) * 128],
                                                 otc[ob],
                                                 start=(ob == 0), stop=(ob == CT - 1))
                            ot = outsb.tile([128, cs], F32, tag="out", name="ot")
                            nc.vector.scalar_tensor_tensor(
                                out=ot, in0=py, scalar=pbt[c], in1=xr,
                                op0=ALU.add, op1=ALU.add)
                            nc.sync.dma_start(
                                out=out_d[b, c * 128:(c + 1) * 128, ncs], in_=ot)
    return nc


_CACHE = {}


def host_inputs(norm_w, norm_b, qkv_w, qkv_b, proj_w, proj_b):
    """Host-side weight preprocessing -> the kernel's shared input tensors."""
    bf = ml_dtypes.bfloat16
    # V-part column permutation: store V as [d*8+g] so the AV multiply reads
    # both operands at unit stride (DVE 2x mode).
    vperm = np.arange(3 * C)
    g_i, d_i = np.meshgrid(np.arange(NH), np.arange(HD), indexing="ij")
    vperm[2 * C:] = 2 * C + (d_i * NH + g_i).reshape(-1)   # old[g*64+d] -> new pos
    inv = np.empty_like(vperm)
    inv[vperm] = np.arange(3 * C)
    wq_p = qkv_w[inv]        # new column j holds old channel inv[j]
    qkvb_p = np.ascontiguousarray(qkv_b[inv])
    wqkvT = np.ascontiguousarray(wq_p.T).astype(bf)           # [C, 3C]
    pwT = np.ascontiguousarray(proj_w.T).astype(bf)           # [C(o), C(c)]
    ident = np.eye(128, dtype=np.float32).astype(bf)
    return dict(wqkvT=wqkvT, pwT=pwT,
                normw=np.asarray(norm_w, np.float32),
                normb=np.asarray(norm_b, np.float32),
                qkvb=qkvb_p, pbeff=np.asarray(proj_b, np.float32),
                ident=ident)


def kernel(x, norm_w, norm_b, qkv_w, qkv_b, proj_w, proj_b):
    x = np.asarray(x, np.float32)
    norm_w = np.asarray(norm_w, np.float32)
    norm_b = np.asarray(norm_b, np.float32)
    qkv_w = np.asarray(qkv_w, np.float32)
    qkv_b = np.asarray(qkv_b, np.float32)
    proj_w = np.asarray(proj_w, np.float32)
    proj_b = np.asarray(proj_b, np.float32)

    qk_bias = bool(np.any(qkv_b != 0))
    key = ("full", qk_bias)
    if key not in _CACHE:
        nc_new = build_kernel(qk_bias=qk_bias)
        _cap_sync_waits(nc_new)   # HW path only; CoreSim rejects bare NoOps
        _CACHE[key] = nc_new
    nc = _CACHE[key]

    shared = host_inputs(norm_w, norm_b, qkv_w, qkv_b, proj_w, proj_b)
    xs = x.reshape(B, C, N)
    in_maps = [dict(x=np.ascontiguousarray(xs[c * NB:(c + 1) * NB]), **shared)
               for c in range(NCORES)]
    res = run_bass_kernel_spmd(nc, in_maps, core_ids=list(range(NCORES)),
                               trace=bool(os.environ.get("KERNEL_TRACE")))
    global LAST_RES
    LAST_RES = res
    out = np.concatenate([res.results[c]["out"] for c in range(NCORES)], axis=0)
    return out.reshape(B, C, HH, WW).astype(np.float32)


LAST_RES = None



# revision 59
# speedup vs baseline: 1.1272x; 1.1272x over previous
"""Trainium2 Bass kernel for nn_AttentionBlock (GroupNorm + per-position
head-axis attention + proj + residual).

Sharding: data-parallel over batch B=16 -> 2 batches per core x 8 cores.

v2 pipeline (per core, per batch):
  1. GroupNorm(32): bn_stats per partition; cross-partition group
     aggregation + broadcast via two tiny PE indicator matmuls (replaces
     the v1 SBUF->SBUF DMA gathers); all per-group scalar math batched
     across the 4 channel tiles in single small DVE ops.
  2. QKV matmul n-major (positions on PSUM partitions); PSUM evicted by
     one ACT copy per block.
  3. Attention processed in PAIRS of 128-position blocks (4D APs) so the
     per-instruction DVE fixed cost (~220ns) is paid half as often:
     u1 = q*k broadcast-multiply (DVE 2x), d-reduction as in-place
     halving add-tree (DVE 2x), softmax (exp on ACT with 1/8 scale
     folded; no max-subtraction -- logits are O(1) by construction),
     AV as broadcast multiply + g-halving tree. The g-tree runs on
     GPSIMD, and the AV multiply runs on GPSIMD for a tunable fraction
     of block-pairs, balancing DVE vs GPSIMD makespan.
  4. O transposed to C-major via PE transposes; proj matmul accumulates
     the residual by an extra identity-weight matmul against a
     host-provided bf16 copy of x; PSUM evicted on ACT with the proj
     bias folded in; DMA out.

Host-side preprocessing: weight transposes + bf16 casts + V-column
permute (AV multiply reads both operands unit-stride so DVE 2x mode
applies) + bf16 x copy + group indicator matrices.

_cap_sync_waits: this walrus build accepts only ONE sync wait per compute
instruction; Tile emits more. The pass hoists excess waits onto
same-engine InstNoOps inserted immediately before the offender.
"""

import contextlib
import os

import numpy as np
import ml_dtypes

import concourse.bass as bass
import concourse.mybir as mybir
import concourse.tile as tile
from concourse.bass_utils import run_bass_kernel_spmd

F32 = mybir.dt.float32
BF16 = mybir.dt.bfloat16

B, C, HH, WW = 16, 512, 64, 64
N = HH * WW            # 4096
NB = 2                 # batches per core
NCORES = 8
NH, HD = 8, 64         # heads, head dim
GROUPS = 32
GSIZE = C // GROUPS    # 16 channels per group
EPS = 1e-5
CT = C // 128          # 4 channel tiles
NBLK = N // 128        # 32 position blocks per batch

AX = mybir.AxisListType
ALU = mybir.AluOpType
ACTF = mybir.ActivationFunctionType


def _bc(t, dims, extra_offset=0):
    """AP over tile/AP `t` with explicit free [step,count] dims (elem units)."""
    return bass.AP(tensor=t.tensor, offset=t.offset + extra_offset,
                   ap=[list(t.ap[0])] + [list(d) for d in dims])


def _cap_sync_waits(nc):
    """Walrus allows at most 1 sync wait per compute instruction; Tile can
    emit more. Hoist the excess onto same-engine InstNoOps inserted
    immediately before the offender."""
    import bass_rust
    n = 0
    for f in nc.m.functions:
        for blk in f.blocks:
            il = blk.instructions
            i = 0
            while i < len(il):
                ins = il[i]
                si = getattr(ins, "sync_info", None)
                if si is not None and si.on_wait and len(si.on_wait) > 1:
                    waits = list(si.on_wait)
                    for w in waits[:-1]:
                        nop = mybir.InstNoOp(name=f"W-abs-{n}", ins=[], outs=[])
                        n += 1
                        nop.engine = ins.engine
                        nop.sync_info = bass_rust.SyncInfo(on_wait=[w],
                                                           on_update=[])
                        il.insert(i, nop)
                        i += 1
                    si.on_wait = waits[-1:]
                i += 1
    return n


KDBG = int(os.environ.get("KDBG", "0"))


def build_kernel(nb=NB, nblk=NBLK, qk_bias=False, gp_u2_frac=0.0,
                 gp_t2=False, gp_t1_frac=0.0, gp_u1_frac=1.0,
                 period_ms=0.0, pq_bufs=2, pmm_bufs=1, qkv_bufs=3,
                 spool_bufs=2, gp_t2_frac=None, gp_stats=False,
                 gp_al=False, gp_t1l1_frac=0.0, tail_t2=0, tail_u2=0):
    n = nblk * 128
    npair = max(1, nblk // 2)      # block pairs (256 positions each)
    cs = min(512, n)               # proj/residual n-chunk
    nc = bass.Bass()

    x_d = nc.dram_tensor("x", [nb, C, n], F32, kind="ExternalInput")
    xbf_d = nc.dram_tensor("xbf", [nb, C, n], BF16, kind="ExternalInput")
    wqkvT_d = nc.dram_tensor("wqkvT", [C, 3 * C], BF16, kind="ExternalInput")
    pwT_d = nc.dram_tensor("pwT", [C, C], BF16, kind="ExternalInput")
    normw_d = nc.dram_tensor("normw", [C], F32, kind="ExternalInput")
    normb_d = nc.dram_tensor("normb", [C], F32, kind="ExternalInput")
    qkvb_d = nc.dram_tensor("qkvb", [3 * C], F32, kind="ExternalInput")
    pbeff_d = nc.dram_tensor("pbeff", [C], F32, kind="ExternalInput")
    ident_d = nc.dram_tensor("ident", [128, 128], BF16, kind="ExternalInput")
    ind_d = nc.dram_tensor("ind", [128, 8], BF16, kind="ExternalInput")
    indT_d = nc.dram_tensor("indT", [8, 128], BF16, kind="ExternalInput")
    out_d = nc.dram_tensor("out", [nb, C, n], F32, kind="ExternalOutput")

    with tile.TileContext(nc) as tc:
        def tt(eng, out, in0, in1, op):
            # (gpsimd scalar_tensor_tensor is priced better by the cost model
            # but does not compile for the Pool engine -- plain TT only)
            eng.tensor_tensor(out=out, in0=in0, in1=in1, op=op)

        def at(slot):
            """Virtual-clock stamp (scheduling-only; TimelineSim/HW replay is
            semaphore-timed). Shapes each engine's static FIFO order so ops
            that wait long never sit ahead of ops that are ready."""
            if period_ms <= 0:
                return contextlib.nullcontext()
            return tc.tile_wait_until(max(0.0, slot) * period_ms)

        with (
            tc.tile_pool(name="consts", bufs=1) as consts,
            tc.tile_pool(name="xpool", bufs=4) as xpool,
            tc.tile_pool(name="hlo", bufs=2) as hlo,
            tc.tile_pool(name="hhi", bufs=1) as hhi,
            tc.tile_pool(name="xbfp", bufs=2) as xbfp,
            tc.tile_pool(name="otr", bufs=2) as otr,
            tc.tile_pool(name="stats", bufs=2) as stats,
            tc.tile_pool(name="scb", bufs=2) as scb,
            tc.tile_pool(name="qkvsb", bufs=qkv_bufs) as qkvsb,
            tc.tile_pool(name="upool", bufs=4) as upool,
            tc.tile_pool(name="spool", bufs=spool_bufs) as spool,
            tc.tile_pool(name="outsb", bufs=3) as outsb,
            tc.tile_pool(name="pqkv", bufs=pq_bufs, space="PSUM") as pqkv,
            tc.tile_pool(name="pmm", bufs=pmm_bufs, space="PSUM") as pmm,
        ):
            # ---- constants / weights in SBUF ----
            wqkvT = []
            for c in range(CT):
                t = consts.tile([128, 3 * C], BF16, tag=f"wq{c}")
                nc.sync.dma_start(out=t, in_=wqkvT_d[c * 128:(c + 1) * 128, :])
                wqkvT.append(t)
            pwT = []
            for o in range(CT):
                t = consts.tile([128, C], BF16, tag=f"pw{o}")
                nc.sync.dma_start(out=t, in_=pwT_d[o * 128:(o + 1) * 128, :])
                pwT.append(t)
            ident = consts.tile([128, 128], BF16, tag="ident")
            nc.sync.dma_start(out=ident, in_=ident_d[:, :])
            ind = consts.tile([128, 8], BF16, tag="ind")
            nc.sync.dma_start(out=ind, in_=ind_d[:, :])
            indT = consts.tile([8, 128], BF16, tag="indT")
            nc.sync.dma_start(out=indT, in_=indT_d[:, :])
            nwt, nbt, pbt = [], [], []
            for c in range(CT):
                sl = slice(c * 128, (c + 1) * 128)
                t1 = consts.tile([128, 1], F32, tag=f"nw{c}")
                nc.sync.dma_start(out=t1, in_=normw_d[sl].rearrange("(p u) -> p u", u=1))
                nwt.append(t1)
                t2 = consts.tile([128, 1], F32, tag=f"nb{c}")
                nc.sync.dma_start(out=t2, in_=normb_d[sl].rearrange("(p u) -> p u", u=1))
                nbt.append(t2)
                t3 = consts.tile([128, 1], F32, tag=f"pb{c}")
                nc.sync.dma_start(out=t3, in_=pbeff_d[sl].rearrange("(p u) -> p u", u=1))
                pbt.append(t3)
            epst = consts.tile([8, 1], F32, tag="eps")
            nc.vector.memset(epst, 256.0 * EPS)
            qkbias = None
            if qk_bias:
                qkbias = consts.tile([128, 3 * C], F32, tag="qkb")
                nc.sync.dma_start(
                    out=qkbias,
                    in_=bass.AP(tensor=qkvb_d.ap().tensor, offset=0,
                                ap=[[0, 128], [1, 3 * C]]))

            def emit_norm(b, g0):
                """GroupNorm scales/biases + normalized h for batch b.

                x is streamed twice in [128, n/2] chunks (stats pass, then
                apply pass) so it never needs full SBUF residency."""
                nh2 = n // 2
                nsub = max(1, n // 512)
                # --- stats pass: stream x chunks, bn_stats into st[c] ---
                st_c = []
                sd = nc.vector.BN_STATS_DIM
                for c in range(CT):
                    st = stats.tile([128, nsub, sd], F32, tag=f"bnst{c}",
                                    name=f"bnst{c}")
                    st_c.append(st)
                for c in range(CT):
                    for half in range(2):
                        hs = slice(half * nh2, (half + 1) * nh2)
                        with at(g0 - 8 + (c * 2 + half) * 0.5):
                            t = xpool.tile([128, nh2], F32, tag="xs", name="xs")
                            nc.sync.dma_start(
                                out=t, in_=x_d[b, c * 128:(c + 1) * 128, hs])
                            xv = t.rearrange("p (s f) -> p s f", s=nsub // 2)
                            seng = nc.gpsimd if gp_stats else nc.vector
                            for s in range(nsub // 2):
                                seng.bn_stats(
                                    out=st_c[c][:, half * (nsub // 2) + s, :],
                                    in_=xv[:, s, :])
                # per-channel running stats -> st2all [128, (ct,2)] bf16
                ctx_norm = at(g0 - 4)
                ctx_norm.__enter__()
                st2all = stats.tile([128, 2 * CT], BF16, tag="st2all",
                                    name="st2all")
                for c in range(CT):
                    mv = stats.tile([128, nc.vector.BN_AGGR_DIM], F32, tag="bnmv")
                    nc.vector.bn_aggr(out=mv, in_=st_c[c])
                    # col0 = mean, col1 = E[x^2] = var + mean^2
                    nc.vector.tensor_copy(out=st2all[:, 2 * c:2 * c + 1],
                                          in_=mv[:, 0:1])
                    nc.vector.scalar_tensor_tensor(
                        out=st2all[:, 2 * c + 1:2 * c + 2], in0=mv[:, 0:1],
                        scalar=mv[:, 0:1], in1=mv[:, 1:2],
                        op0=ALU.mult, op1=ALU.add)
                if KDBG == 2:
                    sc_t, bi_t = [], []
                    for c in range(CT):
                        sc = scb.tile([128, 1], F32, tag=f"sc{c}", name=f"sc{c}")
                        bi = scb.tile([128, 1], F32, tag=f"bi{c}", name=f"bi{c}")
                        nc.vector.memset(sc, 1.0)
                        nc.vector.memset(bi, 0.0)
                        sc_t.append(sc)
                        bi_t.append(bi)
                    ctx_norm.__exit__(None, None, None)
                    ht = [[], []]
                    for half in range(2):
                        hp = hlo if half == 0 else hhi
                        hs = slice(half * nh2, (half + 1) * nh2)
                        for c in range(CT):
                            with at(g0 - 2.5 + (half * CT + c) * 0.25):
                                xa = xpool.tile([128, nh2], F32, tag="xs",
                                                name="xa")
                                nc.sync.dma_start(
                                    out=xa,
                                    in_=x_d[b, c * 128:(c + 1) * 128, hs])
                                t = hp.tile([128, nh2], BF16,
                                            tag=f"h{half}_{c}",
                                            name=f"h{half}_{c}")
                                nc.scalar.activation(out=t, in_=xa,
                                                     func=ACTF.Identity,
                                                     bias=bi_t[c],
                                                     scale=sc_t[c])
                            ht[half].append(t)
                    return ht
                # group aggregation: psum [8, (ct,2)] = sum over 16 channels
                # (full-size py-shaped tile so the PSUM tag stays uniform)
                pg_t = pmm.tile([128, 512], F32, tag="py", name="pg")
                pg = pg_t[0:8, 0:2 * CT]
                nc.tensor.matmul(pg, ind, st2all, start=True, stop=True)
                s8 = stats.tile([8, 2 * CT], F32, tag="s8", name="s8")
                nc.vector.tensor_copy(out=s8, in_=pg)
                # group math on 8 partitions, batched over ct via strided APs
                sm = _bc(s8, [(2, CT)])                  # sum of means
                se = _bc(s8, [(2, CT)], extra_offset=1)  # sum of E[x^2]
                m2 = stats.tile([8, CT], F32, tag="m2", name="m2")
                nc.vector.tensor_mul(m2, sm, sm)
                v256 = stats.tile([8, CT], F32, tag="v256", name="v256")
                nc.vector.scalar_tensor_tensor(
                    out=v256, in0=se, scalar=16.0, in1=m2,
                    op0=ALU.mult, op1=ALU.subtract)
                # rstd/16 = 1/sqrt(256(var+eps))
                sg = stats.tile([8, CT], F32, tag="sg", name="sg")
                nc.scalar.activation(out=sg, in_=v256, func=ACTF.Sqrt,
                                     scale=1.0, bias=epst)
                rg = stats.tile([8, CT], F32, tag="rg", name="rg")
                nc.vector.reciprocal(out=rg, in_=sg)
                # broadcast payload [8, (ct,2)] bf16: col0=rstd, col1=mean
                b8 = stats.tile([8, 2 * CT], BF16, tag="b8", name="b8")
                nc.vector.tensor_scalar(
                    out=_bc(b8, [(2, CT)]), in0=rg, scalar1=16.0,
                    scalar2=None, op0=ALU.mult)
                nc.vector.tensor_scalar(
                    out=_bc(b8, [(2, CT)], extra_offset=1), in0=sm,
                    scalar1=1.0 / 16.0, scalar2=None, op0=ALU.mult)
                pb_t = pmm.tile([128, 512], F32, tag="py", name="pb128")
                pb128 = pb_t[:, 0:2 * CT]
                nc.tensor.matmul(pb128, indT, b8, start=True, stop=True)
                c2 = scb.tile([128, 2 * CT], F32, tag="c2", name="c2")
                nc.vector.tensor_copy(out=c2, in_=pb128)
                # sc[ct] = rstd * nw ; bi[ct] = nb - mean*sc
                sc_t, bi_t = [], []
                for c in range(CT):
                    sc = scb.tile([128, 1], F32, tag=f"sc{c}", name=f"sc{c}")
                    bi = scb.tile([128, 1], F32, tag=f"bi{c}", name=f"bi{c}")
                    nc.vector.tensor_mul(sc, c2[:, 2 * c:2 * c + 1], nwt[c])
                    tmp = stats.tile([128, 1], F32, tag="tmp")
                    nc.vector.tensor_mul(tmp, c2[:, 2 * c + 1:2 * c + 2], sc)
                    nc.vector.tensor_sub(bi, nbt[c], tmp)
                    sc_t.append(sc)
                    bi_t.append(bi)
                ctx_norm.__exit__(None, None, None)
                # --- apply pass: re-stream x chunks -> normalized h ---
                ht = [[], []]
                for half in range(2):
                    hp = hlo if half == 0 else hhi
                    hs = slice(half * nh2, (half + 1) * nh2)
                    for c in range(CT):
                        with at(g0 - 2.5 + (half * CT + c) * 0.25):
                            xa = xpool.tile([128, nh2], F32, tag="xs", name="xa")
                            nc.sync.dma_start(
                                out=xa, in_=x_d[b, c * 128:(c + 1) * 128, hs])
                            t = hp.tile([128, nh2], BF16, tag=f"h{half}_{c}",
                                        name=f"h{half}_{c}")
                            nc.scalar.activation(out=t, in_=xa,
                                                 func=ACTF.Identity,
                                                 bias=bi_t[c], scale=sc_t[c])
                        ht[half].append(t)
                return ht

            ht_next = None
            for b in range(nb):
                if ht_next is None:
                    ht_next = emit_norm(b, b * npair)
                ht = ht_next
                ht_next = None
                bpc = cs // 128                 # blocks per out-chunk (4)
                otc = None
                pending = None
                pending_av = None
                pending_t2 = None
                pending_out = None
                def emit_qkv(pr):
                    g = b * npair + pr
                    blk0 = 2 * pr
                    qkv = qkvsb.tile([128, 2, 3 * C], BF16, tag="qkv")
                    nh2 = n // 2
                    hb = max(1, nblk // 2)
                    for sub in range(2):
                        blk = blk0 + sub
                        half = min(blk // hb, 1)
                        hslice = slice(blk * 128 - half * nh2,
                                       (blk + 1) * 128 - half * nh2)
                        with at(g - 0.7 + sub * 0.1):
                            p = pqkv.tile([128, 3 * C], F32, tag="pq",
                                          name=f"pq{sub}")
                            for c in range(CT):
                                lhsT = ht[half][c][:, hslice]
                                for oc in range(3):
                                    nc.tensor.matmul(
                                        p[:, oc * 512:(oc + 1) * 512], lhsT,
                                        wqkvT[c][:, oc * 512:(oc + 1) * 512],
                                        start=(c == 0), stop=(c == CT - 1))
                            if qkbias is not None:
                                nc.vector.tensor_add(out=qkv[:, sub, :], in0=p,
                                                     in1=qkbias)
                            else:
                                nc.scalar.copy(out=qkv[:, sub, :], in_=p)
                    return qkv

                qkv_next = emit_qkv(0)
                for pr in range(npair):
                    if pr == npair // 2 and b + 1 < nb:
                        # hoist next batch's GroupNorm into this batch's
                        # midsection so its stats/apply overlap attention
                        ht_next = emit_norm(b + 1, (b + 1) * npair)
                    g = b * npair + pr          # global pair slot
                    blk0 = 2 * pr               # first block of the pair
                    qkv = qkv_next

                    # q/k/v APs: qkv [128, (blk, 3C)]
                    QS = 3 * C
                    # ---- logits: u1[(blk,h,g,d)] = q[blk,h,d] * k[blk,g,d]
                    if KDBG == 1:
                        of = outsb.tile([128, 2 * C], F32, tag="out", name="ot")
                        nc.vector.tensor_copy(
                            out=of,
                            in_=_bc(qkv, [(QS, 2), (1, C)], extra_offset=1024))
                        nc.sync.dma_start(
                            out=out_d[b, 0:128, blk0 * 128:(blk0 + 2) * 128]
                                .rearrange("p (a f) -> p a f", a=2)
                                .rearrange("p a f -> p (a f)"),
                            in_=of[:, 0:256])
                        continue
                    u1 = upool.tile([128, 2 * NH * NH * HD], BF16, tag="u")
                    u1eng = nc.gpsimd if (pr % 10) < int(gp_u1_frac * 10 + 1e-6) \
                        else nc.vector
                    if b == 0 and pr == 0:
                        u1eng = nc.vector   # skip GP latency on the very first pair
                    ctx1 = at(g - 0.45); ctx1.__enter__()
                    # per-block 3D APs: the walrus ISA pattern caps compute
                    # APs at 3 free dims and the pair-strided 4D form does
                    # not merge
                    for sub in range(2):
                        tt(u1eng,
                           _bc(u1, [(NH * HD, NH), (HD, NH), (1, HD)],
                               extra_offset=sub * NH * NH * HD),
                           _bc(qkv, [(HD, NH), (0, NH), (1, HD)],
                               extra_offset=sub * QS),
                           _bc(qkv, [(0, NH), (HD, NH), (1, HD)],
                               extra_offset=sub * QS + 512),
                           ALU.mult)
                    ctx1.__exit__(None, None, None)
                    if pr + 1 < npair:
                        qkv_next = emit_qkv(pr + 1)
                    # d-tree: in-place halving on [p, (blk*64), d]
                    t1eng = nc.gpsimd if (pr % 10) < int(gp_t1_frac * 10 + 1e-6) \
                        else nc.vector
                    ctx2 = at(g + 0.05); ctx2.__enter__()
                    u1v = u1.rearrange("p (a d) -> p a d", d=HD)
                    w = HD
                    while w > 2:
                        eng = t1eng
                        if (w == HD and u1eng is nc.gpsimd
                                and (pr % 10) < int(gp_t1l1_frac * 10 + 1e-6)):
                            eng = nc.gpsimd   # L1 rides GP right after u1m
                        tt(eng, u1v[:, :, 0:w // 2], u1v[:, :, 0:w // 2],
                           u1v[:, :, w // 2:w], ALU.add)
                        w //= 2
                    s_l = spool.tile([128, 2 * NH * NH], BF16, tag="s")
                    nc.vector.tensor_tensor(
                        out=s_l.rearrange("p (a u) -> p a u", u=1),
                        in0=u1v[:, :, 0:1], in1=u1v[:, :, 1:2], op=ALU.add)
                    # softmax over g: E = exp(S/8); logits O(1) so no max-sub
                    # (high priority: DVE's d-sum stalls behind ACT's bulk
                    # copies otherwise -- strict per-engine FIFO)
                    ctx2.__exit__(None, None, None)
                    if KDBG == 5:
                        of = outsb.tile([128, 2 * C], F32, tag="out", name="ot")
                        nc.vector.tensor_copy(out=of[:, 0:128], in_=s_l)
                        nc.sync.dma_start(
                            out=out_d[b, 0:128, blk0 * 128:blk0 * 128 + 128],
                            in_=of[:, 0:128])
                        continue
                    e_l = spool.tile([128, 2 * NH * NH], BF16, tag="e")
                    with at(g + 0.38):
                        nc.scalar.activation(out=e_l, in_=s_l, func=ACTF.Exp,
                                             scale=0.125)
                    # deferred AV of the previous pair fills the exp-latency
                    # window on DVE (software pipeline, depth 2); tree2 of the
                    # pair before it is deferred one more slot
                    t2c = pending_av() if pending_av is not None else None
                    if pending_t2 is not None:
                        pending_t2()
                    pending_t2 = t2c
                    ctx3 = at(g + 0.42); ctx3.__enter__()
                    d_l = spool.tile([128, 2 * NH], F32, tag="d")
                    nc.vector.tensor_reduce(
                        out=d_l, in_=e_l.rearrange("p (h g) -> p h g", g=NH),
                        axis=AX.X, op=ALU.add)
                    r_l = spool.tile([128, 2 * NH], F32, tag="r")
                    nc.vector.reciprocal(out=r_l, in_=d_l)
                    a_l = spool.tile([128, 2 * NH * NH], BF16, tag="a")
                    aleng = nc.gpsimd if gp_al else nc.vector
                    tt(aleng, a_l.rearrange("p (h g) -> p h g", g=NH),
                       e_l.rearrange("p (h g) -> p h g", g=NH),
                       _bc(r_l, [(1, 2 * NH), (0, NH)]),
                       ALU.mult)
                    # ---- AV: u2[(blk,h,d,g)] = A[blk,h,g] * V'[blk,d,g]
                    # (V columns host-permuted to [d*8+g]: unit-stride reads)
                    ctx3.__exit__(None, None, None)

                    if KDBG == 3:
                        of = outsb.tile([128, 2 * C], F32, tag="out", name="ot")
                        nc.vector.tensor_copy(out=of[:, 0:128],
                                              in_=_bc(a_l, [(1, 128)]))
                        nc.sync.dma_start(
                            out=out_d[b, 0:128, blk0 * 128:blk0 * 128 + 128],
                            in_=of[:, 0:128])
                        continue

                    def make_av(a_l, qkv, blk0, pr, g):
                        def av():
                            ctx4 = at(g + 1.06)
                            ctx4.__enter__()
                            u2 = upool.tile([128, 2 * NH * HD * NH], BF16,
                                            tag="u")
                            u2eng = nc.gpsimd \
                                if (pr % 10) < int(gp_u2_frac * 10 + 1e-6) \
                                or (b == nb - 1 and pr >= npair - tail_u2) \
                                else nc.vector
                            for sub in range(2):
                                tt(u2eng,
                                   _bc(u2, [(HD * NH, NH), (NH, HD), (1, NH)],
                                       extra_offset=sub * NH * HD * NH),
                                   _bc(a_l, [(NH, NH), (0, HD), (1, NH)],
                                       extra_offset=sub * NH * NH),
                                   _bc(qkv, [(0, NH), (NH, HD), (1, NH)],
                                       extra_offset=sub * QS + 1024),
                                   ALU.mult)
                            ctx4.__exit__(None, None, None)

                            def t2():
                                nonlocal pending, otc
                                ctx5 = at(g + 2.06)
                                ctx5.__enter__()
                                # g-tree: O = sum_g (deferred one more slot so
                                # a GP assignment never blocks the next u1m)
                                f2 = gp_t2_frac if gp_t2_frac is not None \
                                    else (1.0 if gp_t2 else 0.0)
                                t2eng = nc.gpsimd \
                                    if (pr % 10) < int(f2 * 10 + 1e-6) \
                                    or (b == nb - 1 and pr >= npair - tail_t2) \
                                    else nc.vector
                                uv = u2.rearrange("p (a g) -> p a g", g=NH)
                                w = NH
                                while w > 2:
                                    tt(t2eng, uv[:, :, 0:w // 2],
                                       uv[:, :, 0:w // 2],
                                       uv[:, :, w // 2:w], ALU.add)
                                    w //= 2
                                o_l = spool.tile([128, 2 * C], BF16, tag="o")
                                tt(t2eng,
                                   o_l.rearrange("p (a u) -> p a u", u=1),
                                   uv[:, :, 0:1], uv[:, :, 1:2], ALU.add)
                                ctx5.__exit__(None, None, None)
                                if pending is not None:
                                    pending()
                                pending = make_stage2(o_l, blk0, pr, g)
                            return t2
                        return av
                    # ---- stage 2 (transpose + evict + proj), deferred one
                    # pair so ACT/PE FIFOs aren't blocked by waits on the
                    # GPSIMD tree of the current pair
                    def make_stage2(o_l, blk0, pr, g):
                        def stage2():
                            nonlocal otc
                            if blk0 % bpc == 0:
                                otc = otr.tile([128, CT * cs], BF16, tag="otc",
                                               name="otc")
                            for sub in range(2):
                                blk = blk0 + sub
                                with at(g + 2.02 + sub * 0.08):
                                    pt = pmm.tile([128, 512], BF16, tag="pt")
                                    for ob in range(CT):
                                        nc.tensor.transpose(
                                            pt[:, ob * 128:(ob + 1) * 128],
                                            o_l[:, sub * C + ob * 128:
                                                sub * C + (ob + 1) * 128],
                                            ident)
                                    nc.scalar.copy(
                                        out=_bc(otc, [(cs, CT), (1, 128)],
                                                extra_offset=(blk % bpc) * 128),
                                        in_=_bc(pt, [(128, CT), (1, 128)]))
                            if (blk0 + 2) % bpc == 0:
                                nonlocal pending_out
                                j = blk0 // bpc
                                ncs = slice(j * cs, (j + 1) * cs)
                                pys = []
                                for c in range(CT):
                                    with at(g + 2.55 + c * 0.06):
                                        xr = xbfp.tile([128, cs], BF16, tag="xr",
                                                       name="xr")
                                        nc.sync.dma_start(
                                            out=xr,
                                            in_=xbf_d[b, c * 128:(c + 1) * 128,
                                                      ncs])
                                        py = pmm.tile([128, cs], F32, tag="py",
                                                      name="py")
                                        for ob in range(CT):
                                            nc.tensor.matmul(
                                                py,
                                                pwT[ob][:, c * 128:(c + 1) * 128],
                                                otc[:, ob * cs:(ob + 1) * cs],
                                                start=(ob == 0), stop=False)
                                        nc.tensor.matmul(py, ident, xr,
                                                         start=False, stop=True)
                                    pys.append(py)

                                def make_out(pys, ncs, g):
                                    def out_flush():
                                        for c in range(CT):
                                            with at(g + 3.35 + c * 0.06):
                                                ot = outsb.tile(
                                                    [128, cs], F32,
                                                    tag="out", name="ot")
                                                nc.scalar.activation(
                                                    out=ot, in_=pys[c],
                                                    func=ACTF.Identity,
                                                    bias=pbt[c], scale=1.0)
                                                nc.sync.dma_start(
                                                    out=out_d[b,
                                                              c * 128:(c + 1) * 128,
                                                              ncs],
                                                    in_=ot)
                                    return out_flush

                                if pending_out is not None:
                                    pending_out()
                                pending_out = make_out(pys, ncs, g)
                        return stage2

                    pending_av = make_av(a_l, qkv, blk0, pr, g)
                t2c = pending_av() if pending_av is not None else None
                pending_av = None
                if pending_t2 is not None:
                    pending_t2()
                if t2c is not None:
                    t2c()
                pending_t2 = None
                if pending is not None:
                    pending()
                    pending = None
                if pending_out is not None:
                    pending_out()
                    pending_out = None
    return nc


_CACHE = {}


def host_inputs(x, norm_w, norm_b, qkv_w, qkv_b, proj_w, proj_b):
    """Host-side preprocessing -> the kernel's shared input tensors."""
    bf = ml_dtypes.bfloat16
    # V-part column permutation: store V as [d*8+g] so the AV multiply reads
    # both operands at unit stride (DVE 2x mode).
    vperm = np.arange(3 * C)
    g_i, d_i = np.meshgrid(np.arange(NH), np.arange(HD), indexing="ij")
    vperm[2 * C:] = 2 * C + (d_i * NH + g_i).reshape(-1)   # old[g*64+d] -> new
    inv = np.empty_like(vperm)
    inv[vperm] = np.arange(3 * C)
    wq_p = qkv_w[inv]        # new column j holds old channel inv[j]
    qkvb_p = np.ascontiguousarray(qkv_b[inv])
    wqkvT = np.ascontiguousarray(wq_p.T).astype(bf)           # [C, 3C]
    pwT = np.ascontiguousarray(proj_w.T).astype(bf)           # [C(o), C(c)]
    ident = np.eye(128, dtype=np.float32).astype(bf)
    # group indicator: ind[c, g] = 1 if channel c (tile-local) in group g
    ind = np.zeros((128, 8), dtype=np.float32)
    for c in range(128):
        ind[c, c // GSIZE] = 1.0
    indT = np.ascontiguousarray(ind.T)
    return dict(wqkvT=wqkvT, pwT=pwT,
                normw=np.asarray(norm_w, np.float32),
                normb=np.asarray(norm_b, np.float32),
                qkvb=qkvb_p, pbeff=np.asarray(proj_b, np.float32),
                ident=ident, ind=ind.astype(bf), indT=indT.astype(bf))


def kernel(x, norm_w, norm_b, qkv_w, qkv_b, proj_w, proj_b):
    x = np.asarray(x, np.float32)
    norm_w = np.asarray(norm_w, np.float32)
    norm_b = np.asarray(norm_b, np.float32)
    qkv_w = np.asarray(qkv_w, np.float32)
    qkv_b = np.asarray(qkv_b, np.float32)
    proj_w = np.asarray(proj_w, np.float32)
    proj_b = np.asarray(proj_b, np.float32)

    qk_bias = bool(np.any(qkv_b != 0))
    key = ("full", qk_bias)
    if key not in _CACHE:
        nc_new = build_kernel(qk_bias=qk_bias)
        _cap_sync_waits(nc_new)   # HW path only; CoreSim rejects bare NoOps
        _CACHE[key] = nc_new
    nc = _CACHE[key]

    shared = host_inputs(x, norm_w, norm_b, qkv_w, qkv_b, proj_w, proj_b)
    xs = x.reshape(B, C, N)
    xbf = xs.astype(ml_dtypes.bfloat16)
    in_maps = [dict(x=np.ascontiguousarray(xs[c * NB:(c + 1) * NB]),
                    xbf=np.ascontiguousarray(xbf[c * NB:(c + 1) * NB]),
                    **shared)
               for c in range(NCORES)]
    res = run_bass_kernel_spmd(nc, in_maps, core_ids=list(range(NCORES)),
                               trace=bool(os.environ.get("KERNEL_TRACE")))
    global LAST_RES
    LAST_RES = res
    out = np.concatenate([res.results[c]["out"] for c in range(NCORES)], axis=0)
    return out.reshape(B, C, HH, WW).astype(np.float32)


LAST_RES = None


# revision 62
# speedup vs baseline: 1.2021x; 1.0665x over previous
"""Trainium2 Bass kernel for nn_AttentionBlock (GroupNorm + per-position
head-axis attention + proj + residual).

Sharding: data-parallel over batch B=16 -> 2 batches per core x 8 cores.

Per-core pipeline (v2):
  1. GroupNorm(32): x streamed twice in half-row chunks (stats pass +
     apply pass) so it never needs full SBUF residency; cross-partition
     group aggregation + broadcast via two tiny PE indicator matmuls
     (replaces v1's slow SBUF->SBUF DMA gathers); per-group scalar math
     batched across the 4 channel tiles in single small DVE ops. The
     next batch's norm is emitted mid-way through the current batch so
     it fully overlaps attention.
  2. QKV matmul n-major (positions on PSUM partitions); PSUM evicted by
     one ACT copy per block into a [128, 2-block, 3C] bf16 tile.
  3. Attention in PAIRS of 128-position blocks so the ~220ns DVE fixed
     cost per instruction is paid half as often wherever APs allow
     (walrus caps compute APs at 3 free dims, so the q*k and A*V
     broadcast-multiplies are per-block): d-reduction and g-reduction
     as in-place halving add-trees (DVE 2x bf16); softmax with exp on
     ACT (1/8 scale folded, no max-subtraction -- logits are O(1) by
     construction). Software pipelining: GPSIMD runs the u1=q*k multiply
     one pair ahead (one of the two blocks goes to DVE on every 2nd
     pair to balance makespans); the AV multiply of pair p-1 and the
     g-tree of pair p-2 run between tree1(p) and softmax(p) on DVE,
     hiding the ACT exp latency entirely.
  4. O transposed to C-major via PE transposes (evicted by one strided
     ACT copy per block); proj matmul accumulates the residual via an
     extra identity-weight matmul against a host-provided bf16 copy of
     x; PSUM evicted on ACT with the proj bias folded in; DMA out. Both
     stages run 1-2 pairs behind the compute so waits never block the
     ACT/PE FIFOs.

Host-side preprocessing: weight transposes + bf16 casts + V-column
permute to [d*8+g] (AV multiply reads both operands unit-stride so DVE
2x mode applies) + bf16 x copy (residual matmul operand) + group
indicator matrices for the norm matmuls.

_cap_sync_waits: this walrus build accepts only ONE sync wait per compute
instruction; Tile emits more. The pass hoists excess waits onto
same-engine InstNoOps inserted immediately before the offender.
"""

import contextlib
import os

import numpy as np
import ml_dtypes

import concourse.bass as bass
import concourse.mybir as mybir
import concourse.tile as tile
from concourse.bass_utils import run_bass_kernel_spmd

F32 = mybir.dt.float32
BF16 = mybir.dt.bfloat16

B, C, HH, WW = 16, 512, 64, 64
N = HH * WW            # 4096
NB = 2                 # batches per core
NCORES = 8
NH, HD = 8, 64         # heads, head dim
GROUPS = 32
GSIZE = C // GROUPS    # 16 channels per group
EPS = 1e-5
CT = C // 128          # 4 channel tiles
NBLK = N // 128        # 32 position blocks per batch

AX = mybir.AxisListType
ALU = mybir.AluOpType
ACTF = mybir.ActivationFunctionType


def _bc(t, dims, extra_offset=0):
    """AP over tile/AP `t` with explicit free [step,count] dims (elem units)."""
    return bass.AP(tensor=t.tensor, offset=t.offset + extra_offset,
                   ap=[list(t.ap[0])] + [list(d) for d in dims])


def _cap_sync_waits(nc):
    """Walrus allows at most 1 sync wait per compute instruction; Tile can
    emit more. Hoist the excess onto same-engine InstNoOps inserted
    immediately before the offender."""
    import bass_rust
    n = 0
    for f in nc.m.functions:
        for blk in f.blocks:
            il = blk.instructions
            i = 0
            while i < len(il):
                ins = il[i]
                si = getattr(ins, "sync_info", None)
                if si is not None and si.on_wait and len(si.on_wait) > 1:
                    waits = list(si.on_wait)
                    for w in waits[:-1]:
                        nop = mybir.InstNoOp(name=f"W-abs-{n}", ins=[], outs=[])
                        n += 1
                        nop.engine = ins.engine
                        nop.sync_info = bass_rust.SyncInfo(on_wait=[w],
                                                           on_update=[])
                        il.insert(i, nop)
                        i += 1
                    si.on_wait = waits[-1:]
                i += 1
    return n


KDBG = int(os.environ.get("KDBG", "0"))


def build_kernel(nb=NB, nblk=NBLK, qk_bias=False, gp_u2_frac=0.0,
                 gp_t2=False, gp_t1_frac=0.0, gp_u1_frac=1.0,
                 period_ms=0.0, pq_bufs=2, pmm_bufs=1, qkv_bufs=3,
                 spool_bufs=2, gp_t2_frac=None, gp_stats=False,
                 gp_al=False, gp_t1l1_frac=0.0, tail_t2=0, tail_u2=0,
                 u1_dve_sub_every=2):
    n = nblk * 128
    npair = max(1, nblk // 2)      # block pairs (256 positions each)
    cs = min(512, n)               # proj/residual n-chunk
    nc = bass.Bass()

    x_d = nc.dram_tensor("x", [nb, C, n], F32, kind="ExternalInput")
    xbf_d = nc.dram_tensor("xbf", [nb, C, n], BF16, kind="ExternalInput")
    wqkvT_d = nc.dram_tensor("wqkvT", [C, 3 * C], BF16, kind="ExternalInput")
    pwT_d = nc.dram_tensor("pwT", [C, C], BF16, kind="ExternalInput")
    normw_d = nc.dram_tensor("normw", [C], F32, kind="ExternalInput")
    normb_d = nc.dram_tensor("normb", [C], F32, kind="ExternalInput")
    qkvb_d = nc.dram_tensor("qkvb", [3 * C], F32, kind="ExternalInput")
    pbeff_d = nc.dram_tensor("pbeff", [C], F32, kind="ExternalInput")
    ident_d = nc.dram_tensor("ident", [128, 128], BF16, kind="ExternalInput")
    ind_d = nc.dram_tensor("ind", [128, 8], BF16, kind="ExternalInput")
    indT_d = nc.dram_tensor("indT", [8, 128], BF16, kind="ExternalInput")
    out_d = nc.dram_tensor("out", [nb, C, n], F32, kind="ExternalOutput")

    with tile.TileContext(nc) as tc:
        def tt(eng, out, in0, in1, op):
            # (gpsimd scalar_tensor_tensor is priced better by the cost model
            # but does not compile for the Pool engine -- plain TT only)
            eng.tensor_tensor(out=out, in0=in0, in1=in1, op=op)

        def at(slot):
            """Virtual-clock stamp (scheduling-only; TimelineSim/HW replay is
            semaphore-timed). Shapes each engine's static FIFO order so ops
            that wait long never sit ahead of ops that are ready."""
            if period_ms <= 0:
                return contextlib.nullcontext()
            return tc.tile_wait_until(max(0.0, slot) * period_ms)

        with (
            tc.tile_pool(name="consts", bufs=1) as consts,
            tc.tile_pool(name="xpool", bufs=4) as xpool,
            tc.tile_pool(name="hlo", bufs=2) as hlo,
            tc.tile_pool(name="hhi", bufs=1) as hhi,
            tc.tile_pool(name="xbfp", bufs=2) as xbfp,
            tc.tile_pool(name="otr", bufs=2) as otr,
            tc.tile_pool(name="stats", bufs=2) as stats,
            tc.tile_pool(name="scb", bufs=2) as scb,
            tc.tile_pool(name="qkvsb", bufs=qkv_bufs) as qkvsb,
            tc.tile_pool(name="upool", bufs=4) as upool,
            tc.tile_pool(name="spool", bufs=spool_bufs) as spool,
            tc.tile_pool(name="outsb", bufs=3) as outsb,
            tc.tile_pool(name="pqkv", bufs=pq_bufs, space="PSUM") as pqkv,
            tc.tile_pool(name="pmm", bufs=pmm_bufs, space="PSUM") as pmm,
        ):
            # ---- constants / weights in SBUF ----
            wqkvT = []
            for c in range(CT):
                t = consts.tile([128, 3 * C], BF16, tag=f"wq{c}")
                nc.sync.dma_start(out=t, in_=wqkvT_d[c * 128:(c + 1) * 128, :])
                wqkvT.append(t)
            pwT = []
            for o in range(CT):
                t = consts.tile([128, C], BF16, tag=f"pw{o}")
                nc.sync.dma_start(out=t, in_=pwT_d[o * 128:(o + 1) * 128, :])
                pwT.append(t)
            ident = consts.tile([128, 128], BF16, tag="ident")
            nc.sync.dma_start(out=ident, in_=ident_d[:, :])
            ind = consts.tile([128, 8], BF16, tag="ind")
            nc.sync.dma_start(out=ind, in_=ind_d[:, :])
            indT = consts.tile([8, 128], BF16, tag="indT")
            nc.sync.dma_start(out=indT, in_=indT_d[:, :])
            nwt, nbt, pbt = [], [], []
            for c in range(CT):
                sl = slice(c * 128, (c + 1) * 128)
                t1 = consts.tile([128, 1], F32, tag=f"nw{c}")
                nc.sync.dma_start(out=t1, in_=normw_d[sl].rearrange("(p u) -> p u", u=1))
                nwt.append(t1)
                t2 = consts.tile([128, 1], F32, tag=f"nb{c}")
                nc.sync.dma_start(out=t2, in_=normb_d[sl].rearrange("(p u) -> p u", u=1))
                nbt.append(t2)
                t3 = consts.tile([128, 1], F32, tag=f"pb{c}")
                nc.sync.dma_start(out=t3, in_=pbeff_d[sl].rearrange("(p u) -> p u", u=1))
                pbt.append(t3)
            epst = consts.tile([8, 1], F32, tag="eps")
            nc.vector.memset(epst, 256.0 * EPS)
            qkbias = None
            if qk_bias:
                qkbias = consts.tile([128, 3 * C], F32, tag="qkb")
                nc.sync.dma_start(
                    out=qkbias,
                    in_=bass.AP(tensor=qkvb_d.ap().tensor, offset=0,
                                ap=[[0, 128], [1, 3 * C]]))

            def emit_norm(b, g0):
                """GroupNorm scales/biases + normalized h for batch b.

                x is streamed twice in [128, n/2] chunks (stats pass, then
                apply pass) so it never needs full SBUF residency."""
                nh2 = n // 2
                nsub = max(1, n // 512)
                # --- stats pass: stream x chunks, bn_stats into st[c] ---
                st_c = []
                sd = nc.vector.BN_STATS_DIM
                for c in range(CT):
                    st = stats.tile([128, nsub, sd], F32, tag=f"bnst{c}",
                                    name=f"bnst{c}")
                    st_c.append(st)
                for c in range(CT):
                    for half in range(2):
                        hs = slice(half * nh2, (half + 1) * nh2)
                        with at(g0 - 8 + (c * 2 + half) * 0.5):
                            t = xpool.tile([128, nh2], F32, tag="xs", name="xs")
                            nc.sync.dma_start(
                                out=t, in_=x_d[b, c * 128:(c + 1) * 128, hs])
                            xv = t.rearrange("p (s f) -> p s f", s=nsub // 2)
                            seng = nc.gpsimd if gp_stats else nc.vector
                            for s in range(nsub // 2):
                                seng.bn_stats(
                                    out=st_c[c][:, half * (nsub // 2) + s, :],
                                    in_=xv[:, s, :])
                # per-channel running stats -> st2all [128, (ct,2)] bf16
                ctx_norm = at(g0 - 4)
                ctx_norm.__enter__()
                st2all = stats.tile([128, 2 * CT], BF16, tag="st2all",
                                    name="st2all")
                for c in range(CT):
                    mv = stats.tile([128, nc.vector.BN_AGGR_DIM], F32, tag="bnmv")
                    nc.vector.bn_aggr(out=mv, in_=st_c[c])
                    # col0 = mean, col1 = E[x^2] = var + mean^2
                    nc.vector.tensor_copy(out=st2all[:, 2 * c:2 * c + 1],
                                          in_=mv[:, 0:1])
                    nc.vector.scalar_tensor_tensor(
                        out=st2all[:, 2 * c + 1:2 * c + 2], in0=mv[:, 0:1],
                        scalar=mv[:, 0:1], in1=mv[:, 1:2],
                        op0=ALU.mult, op1=ALU.add)
                if KDBG == 2:
                    sc_t, bi_t = [], []
                    for c in range(CT):
                        sc = scb.tile([128, 1], F32, tag=f"sc{c}", name=f"sc{c}")
                        bi = scb.tile([128, 1], F32, tag=f"bi{c}", name=f"bi{c}")
                        nc.vector.memset(sc, 1.0)
                        nc.vector.memset(bi, 0.0)
                        sc_t.append(sc)
                        bi_t.append(bi)
                    ctx_norm.__exit__(None, None, None)
                    ht = [[], []]
                    for half in range(2):
                        hp = hlo if half == 0 else hhi
                        hs = slice(half * nh2, (half + 1) * nh2)
                        for c in range(CT):
                            with at(g0 - 2.5 + (half * CT + c) * 0.25):
                                xa = xpool.tile([128, nh2], F32, tag="xs",
                                                name="xa")
                                nc.sync.dma_start(
                                    out=xa,
                                    in_=x_d[b, c * 128:(c + 1) * 128, hs])
                                t = hp.tile([128, nh2], BF16,
                                            tag=f"h{half}_{c}",
                                            name=f"h{half}_{c}")
                                nc.scalar.activation(out=t, in_=xa,
                                                     func=ACTF.Identity,
                                                     bias=bi_t[c],
                                                     scale=sc_t[c])
                            ht[half].append(t)
                    return ht
                # group aggregation: psum [8, (ct,2)] = sum over 16 channels
                # (full-size py-shaped tile so the PSUM tag stays uniform)
                pg_t = pmm.tile([128, 512], F32, tag="py", name="pg")
                pg = pg_t[0:8, 0:2 * CT]
                nc.tensor.matmul(pg, ind, st2all, start=True, stop=True)
                s8 = stats.tile([8, 2 * CT], F32, tag="s8", name="s8")
                nc.vector.tensor_copy(out=s8, in_=pg)
                # group math on 8 partitions, batched over ct via strided APs
                sm = _bc(s8, [(2, CT)])                  # sum of means
                se = _bc(s8, [(2, CT)], extra_offset=1)  # sum of E[x^2]
                m2 = stats.tile([8, CT], F32, tag="m2", name="m2")
                nc.vector.tensor_mul(m2, sm, sm)
                v256 = stats.tile([8, CT], F32, tag="v256", name="v256")
                nc.vector.scalar_tensor_tensor(
                    out=v256, in0=se, scalar=16.0, in1=m2,
                    op0=ALU.mult, op1=ALU.subtract)
                # rstd/16 = 1/sqrt(256(var+eps))
                sg = stats.tile([8, CT], F32, tag="sg", name="sg")
                nc.scalar.activation(out=sg, in_=v256, func=ACTF.Sqrt,
                                     scale=1.0, bias=epst)
                rg = stats.tile([8, CT], F32, tag="rg", name="rg")
                nc.vector.reciprocal(out=rg, in_=sg)
                # broadcast payload [8, (ct,2)] bf16: col0=rstd, col1=mean
                b8 = stats.tile([8, 2 * CT], BF16, tag="b8", name="b8")
                nc.vector.tensor_scalar(
                    out=_bc(b8, [(2, CT)]), in0=rg, scalar1=16.0,
                    scalar2=None, op0=ALU.mult)
                nc.vector.tensor_scalar(
                    out=_bc(b8, [(2, CT)], extra_offset=1), in0=sm,
                    scalar1=1.0 / 16.0, scalar2=None, op0=ALU.mult)
                pb_t = pmm.tile([128, 512], F32, tag="py", name="pb128")
                pb128 = pb_t[:, 0:2 * CT]
                nc.tensor.matmul(pb128, indT, b8, start=True, stop=True)
                c2 = scb.tile([128, 2 * CT], F32, tag="c2", name="c2")
                nc.vector.tensor_copy(out=c2, in_=pb128)
                # sc[ct] = rstd * nw ; bi[ct] = nb - mean*sc
                sc_t, bi_t = [], []
                for c in range(CT):
                    sc = scb.tile([128, 1], F32, tag=f"sc{c}", name=f"sc{c}")
                    bi = scb.tile([128, 1], F32, tag=f"bi{c}", name=f"bi{c}")
                    nc.vector.tensor_mul(sc, c2[:, 2 * c:2 * c + 1], nwt[c])
                    tmp = stats.tile([128, 1], F32, tag="tmp")
                    nc.vector.tensor_mul(tmp, c2[:, 2 * c + 1:2 * c + 2], sc)
                    nc.vector.tensor_sub(bi, nbt[c], tmp)
                    sc_t.append(sc)
                    bi_t.append(bi)
                ctx_norm.__exit__(None, None, None)
                # --- apply pass: re-stream x chunks -> normalized h ---
                ht = [[], []]
                for half in range(2):
                    hp = hlo if half == 0 else hhi
                    hs = slice(half * nh2, (half + 1) * nh2)
                    for c in range(CT):
                        with at(g0 - 2.5 + (half * CT + c) * 0.25):
                            xa = xpool.tile([128, nh2], F32, tag="xs", name="xa")
                            nc.sync.dma_start(
                                out=xa, in_=x_d[b, c * 128:(c + 1) * 128, hs])
                            t = hp.tile([128, nh2], BF16, tag=f"h{half}_{c}",
                                        name=f"h{half}_{c}")
                            nc.scalar.activation(out=t, in_=xa,
                                                 func=ACTF.Identity,
                                                 bias=bi_t[c], scale=sc_t[c])
                        ht[half].append(t)
                return ht

            ht_next = None
            for b in range(nb):
                if ht_next is None:
                    ht_next = emit_norm(b, b * npair)
                ht = ht_next
                ht_next = None
                bpc = cs // 128                 # blocks per out-chunk (4)
                otc = None
                pending = None
                pending_av = None
                pending_t2 = None
                pending_out = None
                def emit_qkv(pr):
                    g = b * npair + pr
                    blk0 = 2 * pr
                    qkv = qkvsb.tile([128, 2, 3 * C], BF16, tag="qkv")
                    nh2 = n // 2
                    hb = max(1, nblk // 2)
                    for sub in range(2):
                        blk = blk0 + sub
                        half = min(blk // hb, 1)
                        hslice = slice(blk * 128 - half * nh2,
                                       (blk + 1) * 128 - half * nh2)
                        with at(g - 0.7 + sub * 0.1):
                            p = pqkv.tile([128, 3 * C], F32, tag="pq",
                                          name=f"pq{sub}")
                            for c in range(CT):
                                lhsT = ht[half][c][:, hslice]
                                for oc in range(3):
                                    nc.tensor.matmul(
                                        p[:, oc * 512:(oc + 1) * 512], lhsT,
                                        wqkvT[c][:, oc * 512:(oc + 1) * 512],
                                        start=(c == 0), stop=(c == CT - 1))
                            if qkbias is not None:
                                nc.vector.tensor_add(out=qkv[:, sub, :], in0=p,
                                                     in1=qkbias)
                            else:
                                nc.scalar.copy(out=qkv[:, sub, :], in_=p)
                    return qkv

                qkv_next = emit_qkv(0)
                for pr in range(npair):
                    if pr == npair // 2 and b + 1 < nb:
                        # hoist next batch's GroupNorm into this batch's
                        # midsection so its stats/apply overlap attention
                        ht_next = emit_norm(b + 1, (b + 1) * npair)
                    g = b * npair + pr          # global pair slot
                    blk0 = 2 * pr               # first block of the pair
                    qkv = qkv_next

                    # q/k/v APs: qkv [128, (blk, 3C)]
                    QS = 3 * C
                    # ---- logits: u1[(blk,h,g,d)] = q[blk,h,d] * k[blk,g,d]
                    if KDBG == 1:
                        of = outsb.tile([128, 2 * C], F32, tag="out", name="ot")
                        nc.vector.tensor_copy(
                            out=of,
                            in_=_bc(qkv, [(QS, 2), (1, C)], extra_offset=1024))
                        nc.sync.dma_start(
                            out=out_d[b, 0:128, blk0 * 128:(blk0 + 2) * 128]
                                .rearrange("p (a f) -> p a f", a=2)
                                .rearrange("p a f -> p (a f)"),
                            in_=of[:, 0:256])
                        continue
                    u1 = upool.tile([128, 2 * NH * NH * HD], BF16, tag="u")
                    u1eng = nc.gpsimd if (pr % 10) < int(gp_u1_frac * 10 + 1e-6) \
                        else nc.vector
                    if b == 0 and pr == 0:
                        u1eng = nc.vector   # skip GP latency on the very first pair
                    ctx1 = at(g - 0.45); ctx1.__enter__()
                    # per-block 3D APs: the walrus ISA pattern caps compute
                    # APs at 3 free dims and the pair-strided 4D form does
                    # not merge
                    for sub in range(2):
                        eng_s = u1eng
                        if (u1_dve_sub_every and sub == 1
                                and pr % u1_dve_sub_every == 0):
                            eng_s = nc.vector
                        tt(eng_s,
                           _bc(u1, [(NH * HD, NH), (HD, NH), (1, HD)],
                               extra_offset=sub * NH * NH * HD),
                           _bc(qkv, [(HD, NH), (0, NH), (1, HD)],
                               extra_offset=sub * QS),
                           _bc(qkv, [(0, NH), (HD, NH), (1, HD)],
                               extra_offset=sub * QS + 512),
                           ALU.mult)
                    ctx1.__exit__(None, None, None)
                    if pr + 1 < npair:
                        qkv_next = emit_qkv(pr + 1)
                    # d-tree: in-place halving on [p, (blk*64), d]
                    t1eng = nc.gpsimd if (pr % 10) < int(gp_t1_frac * 10 + 1e-6) \
                        else nc.vector
                    ctx2 = at(g + 0.05); ctx2.__enter__()
                    u1v = u1.rearrange("p (a d) -> p a d", d=HD)
                    w = HD
                    while w > 2:
                        eng = t1eng
                        if (w == HD and u1eng is nc.gpsimd
                                and (pr % 10) < int(gp_t1l1_frac * 10 + 1e-6)):
                            eng = nc.gpsimd   # L1 rides GP right after u1m
                        tt(eng, u1v[:, :, 0:w // 2], u1v[:, :, 0:w // 2],
                           u1v[:, :, w // 2:w], ALU.add)
                        w //= 2
                    s_l = spool.tile([128, 2 * NH * NH], BF16, tag="s")
                    nc.vector.tensor_tensor(
                        out=s_l.rearrange("p (a u) -> p a u", u=1),
                        in0=u1v[:, :, 0:1], in1=u1v[:, :, 1:2], op=ALU.add)
                    # softmax over g: E = exp(S/8); logits O(1) so no max-sub
                    # (high priority: DVE's d-sum stalls behind ACT's bulk
                    # copies otherwise -- strict per-engine FIFO)
                    ctx2.__exit__(None, None, None)
                    if KDBG == 5:
                        of = outsb.tile([128, 2 * C], F32, tag="out", name="ot")
                        nc.vector.tensor_copy(out=of[:, 0:128], in_=s_l)
                        nc.sync.dma_start(
                            out=out_d[b, 0:128, blk0 * 128:blk0 * 128 + 128],
                            in_=of[:, 0:128])
                        continue
                    e_l = spool.tile([128, 2 * NH * NH], BF16, tag="e")
                    with at(g + 0.38):
                        nc.scalar.activation(out=e_l, in_=s_l, func=ACTF.Exp,
                                             scale=0.125)
                    # deferred AV of the previous pair fills the exp-latency
                    # window on DVE (software pipeline, depth 2); tree2 of the
                    # pair before it is deferred one more slot
                    t2c = pending_av() if pending_av is not None else None
                    if pending_t2 is not None:
                        pending_t2()
                    pending_t2 = t2c
                    ctx3 = at(g + 0.42); ctx3.__enter__()
                    d_l = spool.tile([128, 2 * NH], F32, tag="d")
                    nc.vector.tensor_reduce(
                        out=d_l, in_=e_l.rearrange("p (h g) -> p h g", g=NH),
                        axis=AX.X, op=ALU.add)
                    r_l = spool.tile([128, 2 * NH], F32, tag="r")
                    nc.vector.reciprocal(out=r_l, in_=d_l)
                    a_l = spool.tile([128, 2 * NH * NH], BF16, tag="a")
                    aleng = nc.gpsimd if gp_al else nc.vector
                    tt(aleng, a_l.rearrange("p (h g) -> p h g", g=NH),
                       e_l.rearrange("p (h g) -> p h g", g=NH),
                       _bc(r_l, [(1, 2 * NH), (0, NH)]),
                       ALU.mult)
                    # ---- AV: u2[(blk,h,d,g)] = A[blk,h,g] * V'[blk,d,g]
                    # (V columns host-permuted to [d*8+g]: unit-stride reads)
                    ctx3.__exit__(None, None, None)

                    if KDBG == 3:
                        of = outsb.tile([128, 2 * C], F32, tag="out", name="ot")
                        nc.vector.tensor_copy(out=of[:, 0:128],
                                              in_=_bc(a_l, [(1, 128)]))
                        nc.sync.dma_start(
                            out=out_d[b, 0:128, blk0 * 128:blk0 * 128 + 128],
                            in_=of[:, 0:128])
                        continue

                    def make_av(a_l, qkv, blk0, pr, g):
                        def av():
                            ctx4 = at(g + 1.06)
                            ctx4.__enter__()
                            u2 = upool.tile([128, 2 * NH * HD * NH], BF16,
                                            tag="u")
                            u2eng = nc.gpsimd \
                                if (pr % 10) < int(gp_u2_frac * 10 + 1e-6) \
                                or (b == nb - 1 and pr >= npair - tail_u2) \
                                else nc.vector
                            for sub in range(2):
                                tt(u2eng,
                                   _bc(u2, [(HD * NH, NH), (NH, HD), (1, NH)],
                                       extra_offset=sub * NH * HD * NH),
                                   _bc(a_l, [(NH, NH), (0, HD), (1, NH)],
                                       extra_offset=sub * NH * NH),
                                   _bc(qkv, [(0, NH), (NH, HD), (1, NH)],
                                       extra_offset=sub * QS + 1024),
                                   ALU.mult)
                            ctx4.__exit__(None, None, None)

                            def t2():
                                nonlocal pending, otc
                                ctx5 = at(g + 2.06)
                                ctx5.__enter__()
                                # g-tree: O = sum_g (deferred one more slot so
                                # a GP assignment never blocks the next u1m)
                                f2 = gp_t2_frac if gp_t2_frac is not None \
                                    else (1.0 if gp_t2 else 0.0)
                                t2eng = nc.gpsimd \
                                    if (pr % 10) < int(f2 * 10 + 1e-6) \
                                    or (b == nb - 1 and pr >= npair - tail_t2) \
                                    else nc.vector
                                uv = u2.rearrange("p (a g) -> p a g", g=NH)
                                w = NH
                                while w > 2:
                                    tt(t2eng, uv[:, :, 0:w // 2],
                                       uv[:, :, 0:w // 2],
                                       uv[:, :, w // 2:w], ALU.add)
                                    w //= 2
                                o_l = spool.tile([128, 2 * C], BF16, tag="o")
                                tt(t2eng,
                                   o_l.rearrange("p (a u) -> p a u", u=1),
                                   uv[:, :, 0:1], uv[:, :, 1:2], ALU.add)
                                ctx5.__exit__(None, None, None)
                                if pending is not None:
                                    pending()
                                pending = make_stage2(o_l, blk0, pr, g)
                            return t2
                        return av
                    # ---- stage 2 (transpose + evict + proj), deferred one
                    # pair so ACT/PE FIFOs aren't blocked by waits on the
                    # GPSIMD tree of the current pair
                    def make_stage2(o_l, blk0, pr, g):
                        def stage2():
                            nonlocal otc
                            if blk0 % bpc == 0:
                                otc = otr.tile([128, CT * cs], BF16, tag="otc",
                                               name="otc")
                            for sub in range(2):
                                blk = blk0 + sub
                                with at(g + 2.02 + sub * 0.08):
                                    pt = pmm.tile([128, 512], BF16, tag="pt")
                                    for ob in range(CT):
                                        nc.tensor.transpose(
                                            pt[:, ob * 128:(ob + 1) * 128],
                                            o_l[:, sub * C + ob * 128:
                                                sub * C + (ob + 1) * 128],
                                            ident)
                                    nc.scalar.copy(
                                        out=_bc(otc, [(cs, CT), (1, 128)],
                                                extra_offset=(blk % bpc) * 128),
                                        in_=_bc(pt, [(128, CT), (1, 128)]))
                            if (blk0 + 2) % bpc == 0:
                                nonlocal pending_out
                                j = blk0 // bpc
                                ncs = slice(j * cs, (j + 1) * cs)
                                pys = []
                                for c in range(CT):
                                    with at(g + 2.55 + c * 0.06):
                                        xr = xbfp.tile([128, cs], BF16, tag="xr",
                                                       name="xr")
                                        nc.sync.dma_start(
                                            out=xr,
                                            in_=xbf_d[b, c * 128:(c + 1) * 128,
                                                      ncs])
                                        py = pmm.tile([128, cs], F32, tag="py",
                                                      name="py")
                                        for ob in range(CT):
                                            nc.tensor.matmul(
                                                py,
                                                pwT[ob][:, c * 128:(c + 1) * 128],
                                                otc[:, ob * cs:(ob + 1) * cs],
                                                start=(ob == 0), stop=False)
                                        nc.tensor.matmul(py, ident, xr,
                                                         start=False, stop=True)
                                    pys.append(py)

                                def make_out(pys, ncs, g):
                                    def out_flush():
                                        for c in range(CT):
                                            with at(g + 3.35 + c * 0.06):
                                                ot = outsb.tile(
                                                    [128, cs], F32,
                                                    tag="out", name="ot")
                                                nc.scalar.activation(
                                                    out=ot, in_=pys[c],
                                                    func=ACTF.Identity,
                                                    bias=pbt[c], scale=1.0)
                                                nc.sync.dma_start(
                                                    out=out_d[b,
                                                              c * 128:(c + 1) * 128,
                                                              ncs],
                                                    in_=ot)
                                    return out_flush

                                if pending_out is not None:
                                    pending_out()
                                pending_out = make_out(pys, ncs, g)
                        return stage2

                    pending_av = make_av(a_l, qkv, blk0, pr, g)
                t2c = pending_av() if pending_av is not None else None
                pending_av = None
                if pending_t2 is not None:
                    pending_t2()
                if t2c is not None:
                    t2c()
                pending_t2 = None
                if pending is not None:
                    pending()
                    pending = None
                if pending_out is not None:
                    pending_out()
                    pending_out = None
    return nc


_CACHE = {}


def host_inputs(x, norm_w, norm_b, qkv_w, qkv_b, proj_w, proj_b):
    """Host-side preprocessing -> the kernel's shared input tensors."""
    bf = ml_dtypes.bfloat16
    # V-part column permutation: store V as [d*8+g] so the AV multiply reads
    # both operands at unit stride (DVE 2x mode).
    vperm = np.arange(3 * C)
    g_i, d_i = np.meshgrid(np.arange(NH), np.arange(HD), indexing="ij")
    vperm[2 * C:] = 2 * C + (d_i * NH + g_i).reshape(-1)   # old[g*64+d] -> new
    inv = np.empty_like(vperm)
    inv[vperm] = np.arange(3 * C)
    wq_p = qkv_w[inv]        # new column j holds old channel inv[j]
    qkvb_p = np.ascontiguousarray(qkv_b[inv])
    wqkvT = np.ascontiguousarray(wq_p.T).astype(bf)           # [C, 3C]
    pwT = np.ascontiguousarray(proj_w.T).astype(bf)           # [C(o), C(c)]
    ident = np.eye(128, dtype=np.float32).astype(bf)
    # group indicator: ind[c, g] = 1 if channel c (tile-local) in group g
    ind = np.zeros((128, 8), dtype=np.float32)
    for c in range(128):
        ind[c, c // GSIZE] = 1.0
    indT = np.ascontiguousarray(ind.T)
    return dict(wqkvT=wqkvT, pwT=pwT,
                normw=np.asarray(norm_w, np.float32),
                normb=np.asarray(norm_b, np.float32),
                qkvb=qkvb_p, pbeff=np.asarray(proj_b, np.float32),
                ident=ident, ind=ind.astype(bf), indT=indT.astype(bf))


def kernel(x, norm_w, norm_b, qkv_w, qkv_b, proj_w, proj_b):
    x = np.asarray(x, np.float32)
    norm_w = np.asarray(norm_w, np.float32)
    norm_b = np.asarray(norm_b, np.float32)
    qkv_w = np.asarray(qkv_w, np.float32)
    qkv_b = np.asarray(qkv_b, np.float32)
    proj_w = np.asarray(proj_w, np.float32)
    proj_b = np.asarray(proj_b, np.float32)

    qk_bias = bool(np.any(qkv_b != 0))
    key = ("full", qk_bias)
    if key not in _CACHE:
        nc_new = build_kernel(qk_bias=qk_bias)
        _cap_sync_waits(nc_new)   # HW path only; CoreSim rejects bare NoOps
        _CACHE[key] = nc_new
    nc = _CACHE[key]

    shared = host_inputs(x, norm_w, norm_b, qkv_w, qkv_b, proj_w, proj_b)
    xs = x.reshape(B, C, N)
    xbf = xs.astype(ml_dtypes.bfloat16)
    in_maps = [dict(x=np.ascontiguousarray(xs[c * NB:(c + 1) * NB]),
                    xbf=np.ascontiguousarray(xbf[c * NB:(c + 1) * NB]),
                    **shared)
               for c in range(NCORES)]
    res = run_bass_kernel_spmd(nc, in_maps, core_ids=list(range(NCORES)),
                               trace=bool(os.environ.get("KERNEL_TRACE")))
    global LAST_RES
    LAST_RES = res
    out = np.concatenate([res.results[c]["out"] for c in range(NCORES)], axis=0)
    return out.reshape(B, C, HH, WW).astype(np.float32)


LAST_RES = None


# revision 64
# speedup vs baseline: 1.2091x; 1.0058x over previous
"""Trainium2 Bass kernel for nn_AttentionBlock (GroupNorm + per-position
head-axis attention + proj + residual).

Sharding: data-parallel over batch B=16 -> 2 batches per core x 8 cores.

Per-core pipeline (v2):
  1. GroupNorm(32): x streamed twice in half-row chunks (stats pass +
     apply pass) so it never needs full SBUF residency; cross-partition
     group aggregation + broadcast via two tiny PE indicator matmuls
     (replaces v1's slow SBUF->SBUF DMA gathers); per-group scalar math
     batched across the 4 channel tiles in single small DVE ops. The
     next batch's norm is emitted mid-way through the current batch so
     it fully overlaps attention.
  2. QKV matmul n-major (positions on PSUM partitions); PSUM evicted by
     one ACT copy per block into a [128, 2-block, 3C] bf16 tile.
  3. Attention in PAIRS of 128-position blocks so the ~220ns DVE fixed
     cost per instruction is paid half as often wherever APs allow
     (walrus caps compute APs at 3 free dims, so the q*k and A*V
     broadcast-multiplies are per-block): d-reduction and g-reduction
     as in-place halving add-trees (DVE 2x bf16); softmax with exp on
     ACT (1/8 scale folded, no max-subtraction -- logits are O(1) by
     construction). Software pipelining: GPSIMD runs the u1=q*k multiply
     one pair ahead (one of the two blocks goes to DVE on every 2nd
     pair to balance makespans); the AV multiply of pair p-1 and the
     g-tree of pair p-2 run between tree1(p) and softmax(p) on DVE,
     hiding the ACT exp latency entirely.
  4. O transposed to C-major via PE transposes (evicted by one strided
     ACT copy per block); proj matmul accumulates the residual via an
     extra identity-weight matmul against a host-provided bf16 copy of
     x; PSUM evicted on ACT with the proj bias folded in; DMA out. Both
     stages run 1-2 pairs behind the compute so waits never block the
     ACT/PE FIFOs.

Host-side preprocessing: weight transposes + bf16 casts + V-column
permute to [d*8+g] (AV multiply reads both operands unit-stride so DVE
2x mode applies) + bf16 x copy (residual matmul operand) + group
indicator matrices for the norm matmuls.

_cap_sync_waits: this walrus build accepts only ONE sync wait per compute
instruction; Tile emits more. The pass hoists excess waits onto
same-engine InstNoOps inserted immediately before the offender.
"""

import contextlib
import os

import numpy as np
import ml_dtypes

import concourse.bass as bass
import concourse.mybir as mybir
import concourse.tile as tile
from concourse.bass_utils import run_bass_kernel_spmd

F32 = mybir.dt.float32
BF16 = mybir.dt.bfloat16

B, C, HH, WW = 16, 512, 64, 64
N = HH * WW            # 4096
NB = 2                 # batches per core
NCORES = 8
NH, HD = 8, 64         # heads, head dim
GROUPS = 32
GSIZE = C // GROUPS    # 16 channels per group
EPS = 1e-5
CT = C // 128          # 4 channel tiles
NBLK = N // 128        # 32 position blocks per batch

AX = mybir.AxisListType
ALU = mybir.AluOpType
ACTF = mybir.ActivationFunctionType


def _bc(t, dims, extra_offset=0):
    """AP over tile/AP `t` with explicit free [step,count] dims (elem units)."""
    return bass.AP(tensor=t.tensor, offset=t.offset + extra_offset,
                   ap=[list(t.ap[0])] + [list(d) for d in dims])


def _cap_sync_waits(nc):
    """Walrus allows at most 1 sync wait per compute instruction; Tile can
    emit more. Hoist the excess onto same-engine InstNoOps inserted
    immediately before the offender."""
    import bass_rust
    n = 0
    for f in nc.m.functions:
        for blk in f.blocks:
            il = blk.instructions
            i = 0
            while i < len(il):
                ins = il[i]
                si = getattr(ins, "sync_info", None)
                if si is not None and si.on_wait and len(si.on_wait) > 1:
                    waits = list(si.on_wait)
                    for w in waits[:-1]:
                        nop = mybir.InstNoOp(name=f"W-abs-{n}", ins=[], outs=[])
                        n += 1
                        nop.engine = ins.engine
                        nop.sync_info = bass_rust.SyncInfo(on_wait=[w],
                                                           on_update=[])
                        il.insert(i, nop)
                        i += 1
                    si.on_wait = waits[-1:]
                i += 1
    return n


KDBG = int(os.environ.get("KDBG", "0"))


def build_kernel(nb=NB, nblk=NBLK, qk_bias=False, gp_u2_frac=0.0,
                 gp_t2=False, gp_t1_frac=0.0, gp_u1_frac=1.0,
                 period_ms=0.0, pq_bufs=2, pmm_bufs=1, qkv_bufs=3,
                 spool_bufs=2, gp_t2_frac=None, gp_stats=False,
                 gp_al=False, gp_t1l1_frac=0.0, tail_t2=0, tail_u2=0,
                 u1_dve_sub_every=-3):
    n = nblk * 128
    npair = max(1, nblk // 2)      # block pairs (256 positions each)
    cs = min(512, n)               # proj/residual n-chunk
    nc = bass.Bass()

    x_d = nc.dram_tensor("x", [nb, C, n], F32, kind="ExternalInput")
    xbf_d = nc.dram_tensor("xbf", [nb, C, n], BF16, kind="ExternalInput")
    wqkvT_d = nc.dram_tensor("wqkvT", [C, 3 * C], BF16, kind="ExternalInput")
    pwT_d = nc.dram_tensor("pwT", [C, C], BF16, kind="ExternalInput")
    normw_d = nc.dram_tensor("normw", [C], F32, kind="ExternalInput")
    normb_d = nc.dram_tensor("normb", [C], F32, kind="ExternalInput")
    qkvb_d = nc.dram_tensor("qkvb", [3 * C], F32, kind="ExternalInput")
    pbeff_d = nc.dram_tensor("pbeff", [C], F32, kind="ExternalInput")
    ident_d = nc.dram_tensor("ident", [128, 128], BF16, kind="ExternalInput")
    ind_d = nc.dram_tensor("ind", [128, 8], BF16, kind="ExternalInput")
    indT_d = nc.dram_tensor("indT", [8, 128], BF16, kind="ExternalInput")
    out_d = nc.dram_tensor("out", [nb, C, n], F32, kind="ExternalOutput")

    with tile.TileContext(nc) as tc:
        def tt(eng, out, in0, in1, op):
            # (gpsimd scalar_tensor_tensor is priced better by the cost model
            # but does not compile for the Pool engine -- plain TT only)
            eng.tensor_tensor(out=out, in0=in0, in1=in1, op=op)

        def at(slot):
            """Virtual-clock stamp (scheduling-only; TimelineSim/HW replay is
            semaphore-timed). Shapes each engine's static FIFO order so ops
            that wait long never sit ahead of ops that are ready."""
            if period_ms <= 0:
                return contextlib.nullcontext()
            return tc.tile_wait_until(max(0.0, slot) * period_ms)

        with (
            tc.tile_pool(name="consts", bufs=1) as consts,
            tc.tile_pool(name="xpool", bufs=4) as xpool,
            tc.tile_pool(name="hlo", bufs=2) as hlo,
            tc.tile_pool(name="hhi", bufs=1) as hhi,
            tc.tile_pool(name="xbfp", bufs=2) as xbfp,
            tc.tile_pool(name="otr", bufs=2) as otr,
            tc.tile_pool(name="stats", bufs=2) as stats,
            tc.tile_pool(name="scb", bufs=2) as scb,
            tc.tile_pool(name="qkvsb", bufs=qkv_bufs) as qkvsb,
            tc.tile_pool(name="upool", bufs=4) as upool,
            tc.tile_pool(name="spool", bufs=spool_bufs) as spool,
            tc.tile_pool(name="outsb", bufs=3) as outsb,
            tc.tile_pool(name="pqkv", bufs=pq_bufs, space="PSUM") as pqkv,
            tc.tile_pool(name="pmm", bufs=pmm_bufs, space="PSUM") as pmm,
        ):
            # ---- constants / weights in SBUF ----
            wqkvT = []
            for c in range(CT):
                t = consts.tile([128, 3 * C], BF16, tag=f"wq{c}")
                nc.sync.dma_start(out=t, in_=wqkvT_d[c * 128:(c + 1) * 128, :])
                wqkvT.append(t)
            pwT = []
            for o in range(CT):
                t = consts.tile([128, C], BF16, tag=f"pw{o}")
                nc.sync.dma_start(out=t, in_=pwT_d[o * 128:(o + 1) * 128, :])
                pwT.append(t)
            ident = consts.tile([128, 128], BF16, tag="ident")
            nc.sync.dma_start(out=ident, in_=ident_d[:, :])
            ind = consts.tile([128, 8], BF16, tag="ind")
            nc.sync.dma_start(out=ind, in_=ind_d[:, :])
            indT = consts.tile([8, 128], BF16, tag="indT")
            nc.sync.dma_start(out=indT, in_=indT_d[:, :])
            nwt, nbt, pbt = [], [], []
            for c in range(CT):
                sl = slice(c * 128, (c + 1) * 128)
                t1 = consts.tile([128, 1], F32, tag=f"nw{c}")
                nc.sync.dma_start(out=t1, in_=normw_d[sl].rearrange("(p u) -> p u", u=1))
                nwt.append(t1)
                t2 = consts.tile([128, 1], F32, tag=f"nb{c}")
                nc.sync.dma_start(out=t2, in_=normb_d[sl].rearrange("(p u) -> p u", u=1))
                nbt.append(t2)
                t3 = consts.tile([128, 1], F32, tag=f"pb{c}")
                nc.sync.dma_start(out=t3, in_=pbeff_d[sl].rearrange("(p u) -> p u", u=1))
                pbt.append(t3)
            epst = consts.tile([8, 1], F32, tag="eps")
            nc.vector.memset(epst, 256.0 * EPS)
            qkbias = None
            if qk_bias:
                qkbias = consts.tile([128, 3 * C], F32, tag="qkb")
                nc.sync.dma_start(
                    out=qkbias,
                    in_=bass.AP(tensor=qkvb_d.ap().tensor, offset=0,
                                ap=[[0, 128], [1, 3 * C]]))

            def emit_norm(b, g0):
                """GroupNorm scales/biases + normalized h for batch b.

                x is streamed twice in [128, n/2] chunks (stats pass, then
                apply pass) so it never needs full SBUF residency."""
                nh2 = n // 2
                nsub = max(1, n // 512)
                # --- stats pass: stream x chunks, bn_stats into st[c] ---
                st_c = []
                sd = nc.vector.BN_STATS_DIM
                for c in range(CT):
                    st = stats.tile([128, nsub, sd], F32, tag=f"bnst{c}",
                                    name=f"bnst{c}")
                    st_c.append(st)
                for c in range(CT):
                    for half in range(2):
                        hs = slice(half * nh2, (half + 1) * nh2)
                        with at(g0 - 8 + (c * 2 + half) * 0.5):
                            t = xpool.tile([128, nh2], F32, tag="xs", name="xs")
                            nc.sync.dma_start(
                                out=t, in_=x_d[b, c * 128:(c + 1) * 128, hs])
                            xv = t.rearrange("p (s f) -> p s f", s=nsub // 2)
                            seng = nc.gpsimd if gp_stats else nc.vector
                            for s in range(nsub // 2):
                                seng.bn_stats(
                                    out=st_c[c][:, half * (nsub // 2) + s, :],
                                    in_=xv[:, s, :])
                # per-channel running stats -> st2all [128, (ct,2)] bf16
                ctx_norm = at(g0 - 4)
                ctx_norm.__enter__()
                st2all = stats.tile([128, 2 * CT], BF16, tag="st2all",
                                    name="st2all")
                for c in range(CT):
                    mv = stats.tile([128, nc.vector.BN_AGGR_DIM], F32, tag="bnmv")
                    nc.vector.bn_aggr(out=mv, in_=st_c[c])
                    # col0 = mean, col1 = E[x^2] = var + mean^2
                    nc.vector.tensor_copy(out=st2all[:, 2 * c:2 * c + 1],
                                          in_=mv[:, 0:1])
                    nc.vector.scalar_tensor_tensor(
                        out=st2all[:, 2 * c + 1:2 * c + 2], in0=mv[:, 0:1],
                        scalar=mv[:, 0:1], in1=mv[:, 1:2],
                        op0=ALU.mult, op1=ALU.add)
                if KDBG == 2:
                    sc_t, bi_t = [], []
                    for c in range(CT):
                        sc = scb.tile([128, 1], F32, tag=f"sc{c}", name=f"sc{c}")
                        bi = scb.tile([128, 1], F32, tag=f"bi{c}", name=f"bi{c}")
                        nc.vector.memset(sc, 1.0)
                        nc.vector.memset(bi, 0.0)
                        sc_t.append(sc)
                        bi_t.append(bi)
                    ctx_norm.__exit__(None, None, None)
                    ht = [[], []]
                    for half in range(2):
                        hp = hlo if half == 0 else hhi
                        hs = slice(half * nh2, (half + 1) * nh2)
                        for c in range(CT):
                            with at(g0 - 2.5 + (half * CT + c) * 0.25):
                                xa = xpool.tile([128, nh2], F32, tag="xs",
                                                name="xa")
                                nc.sync.dma_start(
                                    out=xa,
                                    in_=x_d[b, c * 128:(c + 1) * 128, hs])
                                t = hp.tile([128, nh2], BF16,
                                            tag=f"h{half}_{c}",
                                            name=f"h{half}_{c}")
                                nc.scalar.activation(out=t, in_=xa,
                                                     func=ACTF.Identity,
                                                     bias=bi_t[c],
                                                     scale=sc_t[c])
                            ht[half].append(t)
                    return ht
                # group aggregation: psum [8, (ct,2)] = sum over 16 channels
                # (full-size py-shaped tile so the PSUM tag stays uniform)
                pg_t = pmm.tile([128, 512], F32, tag="py", name="pg")
                pg = pg_t[0:8, 0:2 * CT]
                nc.tensor.matmul(pg, ind, st2all, start=True, stop=True)
                s8 = stats.tile([8, 2 * CT], F32, tag="s8", name="s8")
                nc.vector.tensor_copy(out=s8, in_=pg)
                # group math on 8 partitions, batched over ct via strided APs
                sm = _bc(s8, [(2, CT)])                  # sum of means
                se = _bc(s8, [(2, CT)], extra_offset=1)  # sum of E[x^2]
                m2 = stats.tile([8, CT], F32, tag="m2", name="m2")
                nc.vector.tensor_mul(m2, sm, sm)
                v256 = stats.tile([8, CT], F32, tag="v256", name="v256")
                nc.vector.scalar_tensor_tensor(
                    out=v256, in0=se, scalar=16.0, in1=m2,
                    op0=ALU.mult, op1=ALU.subtract)
                # rstd/16 = 1/sqrt(256(var+eps))
                sg = stats.tile([8, CT], F32, tag="sg", name="sg")
                nc.scalar.activation(out=sg, in_=v256, func=ACTF.Sqrt,
                                     scale=1.0, bias=epst)
                rg = stats.tile([8, CT], F32, tag="rg", name="rg")
                nc.vector.reciprocal(out=rg, in_=sg)
                # broadcast payload [8, (ct,2)] bf16: col0=rstd, col1=mean
                b8 = stats.tile([8, 2 * CT], BF16, tag="b8", name="b8")
                nc.vector.tensor_scalar(
                    out=_bc(b8, [(2, CT)]), in0=rg, scalar1=16.0,
                    scalar2=None, op0=ALU.mult)
                nc.vector.tensor_scalar(
                    out=_bc(b8, [(2, CT)], extra_offset=1), in0=sm,
                    scalar1=1.0 / 16.0, scalar2=None, op0=ALU.mult)
                pb_t = pmm.tile([128, 512], F32, tag="py", name="pb128")
                pb128 = pb_t[:, 0:2 * CT]
                nc.tensor.matmul(pb128, indT, b8, start=True, stop=True)
                c2 = scb.tile([128, 2 * CT], F32, tag="c2", name="c2")
                nc.vector.tensor_copy(out=c2, in_=pb128)
                # sc[ct] = rstd * nw ; bi[ct] = nb - mean*sc
                sc_t, bi_t = [], []
                for c in range(CT):
                    sc = scb.tile([128, 1], F32, tag=f"sc{c}", name=f"sc{c}")
                    bi = scb.tile([128, 1], F32, tag=f"bi{c}", name=f"bi{c}")
                    nc.vector.tensor_mul(sc, c2[:, 2 * c:2 * c + 1], nwt[c])
                    tmp = stats.tile([128, 1], F32, tag="tmp")
                    nc.vector.tensor_mul(tmp, c2[:, 2 * c + 1:2 * c + 2], sc)
                    nc.vector.tensor_sub(bi, nbt[c], tmp)
                    sc_t.append(sc)
                    bi_t.append(bi)
                ctx_norm.__exit__(None, None, None)
                # --- apply pass: re-stream x chunks -> normalized h ---
                ht = [[], []]
                for half in range(2):
                    hp = hlo if half == 0 else hhi
                    hs = slice(half * nh2, (half + 1) * nh2)
                    for c in range(CT):
                        with at(g0 - 2.5 + (half * CT + c) * 0.25):
                            xa = xpool.tile([128, nh2], F32, tag="xs", name="xa")
                            nc.sync.dma_start(
                                out=xa, in_=x_d[b, c * 128:(c + 1) * 128, hs])
                            t = hp.tile([128, nh2], BF16, tag=f"h{half}_{c}",
                                        name=f"h{half}_{c}")
                            nc.scalar.activation(out=t, in_=xa,
                                                 func=ACTF.Identity,
                                                 bias=bi_t[c], scale=sc_t[c])
                        ht[half].append(t)
                return ht

            ht_next = None
            for b in range(nb):
                if ht_next is None:
                    ht_next = emit_norm(b, b * npair)
                ht = ht_next
                ht_next = None
                bpc = cs // 128                 # blocks per out-chunk (4)
                otc = None
                pending = None
                pending_av = None
                pending_t2 = None
                pending_out = None
                def emit_qkv(pr):
                    g = b * npair + pr
                    blk0 = 2 * pr
                    qkv = qkvsb.tile([128, 2, 3 * C], BF16, tag="qkv")
                    nh2 = n // 2
                    hb = max(1, nblk // 2)
                    for sub in range(2):
                        blk = blk0 + sub
                        half = min(blk // hb, 1)
                        hslice = slice(blk * 128 - half * nh2,
                                       (blk + 1) * 128 - half * nh2)
                        with at(g - 0.7 + sub * 0.1):
                            p = pqkv.tile([128, 3 * C], F32, tag="pq",
                                          name=f"pq{sub}")
                            for c in range(CT):
                                lhsT = ht[half][c][:, hslice]
                                for oc in range(3):
                                    nc.tensor.matmul(
                                        p[:, oc * 512:(oc + 1) * 512], lhsT,
                                        wqkvT[c][:, oc * 512:(oc + 1) * 512],
                                        start=(c == 0), stop=(c == CT - 1))
                            if qkbias is not None:
                                nc.vector.tensor_add(out=qkv[:, sub, :], in0=p,
                                                     in1=qkbias)
                            else:
                                nc.scalar.copy(out=qkv[:, sub, :], in_=p)
                    return qkv

                qkv_next = emit_qkv(0)
                for pr in range(npair):
                    if pr == npair // 2 and b + 1 < nb:
                        # hoist next batch's GroupNorm into this batch's
                        # midsection so its stats/apply overlap attention
                        ht_next = emit_norm(b + 1, (b + 1) * npair)
                    g = b * npair + pr          # global pair slot
                    blk0 = 2 * pr               # first block of the pair
                    qkv = qkv_next

                    # q/k/v APs: qkv [128, (blk, 3C)]
                    QS = 3 * C
                    # ---- logits: u1[(blk,h,g,d)] = q[blk,h,d] * k[blk,g,d]
                    if KDBG == 1:
                        of = outsb.tile([128, 2 * C], F32, tag="out", name="ot")
                        nc.vector.tensor_copy(
                            out=of,
                            in_=_bc(qkv, [(QS, 2), (1, C)], extra_offset=1024))
                        nc.sync.dma_start(
                            out=out_d[b, 0:128, blk0 * 128:(blk0 + 2) * 128]
                                .rearrange("p (a f) -> p a f", a=2)
                                .rearrange("p a f -> p (a f)"),
                            in_=of[:, 0:256])
                        continue
                    u1 = upool.tile([128, 2 * NH * NH * HD], BF16, tag="u")
                    u1eng = nc.gpsimd if (pr % 10) < int(gp_u1_frac * 10 + 1e-6) \
                        else nc.vector
                    if b == 0 and pr == 0:
                        u1eng = nc.vector   # skip GP latency on the very first pair
                    ctx1 = at(g - 0.45); ctx1.__enter__()
                    # per-block 3D APs: the walrus ISA pattern caps compute
                    # APs at 3 free dims and the pair-strided 4D form does
                    # not merge
                    for sub in range(2):
                        eng_s = u1eng
                        if u1_dve_sub_every and sub == 1:
                            k = u1_dve_sub_every
                            hit = (pr % k == 0) if k > 0 else (pr % (-k) != 0)
                            if hit:
                                eng_s = nc.vector
                        tt(eng_s,
                           _bc(u1, [(NH * HD, NH), (HD, NH), (1, HD)],
                               extra_offset=sub * NH * NH * HD),
                           _bc(qkv, [(HD, NH), (0, NH), (1, HD)],
                               extra_offset=sub * QS),
                           _bc(qkv, [(0, NH), (HD, NH), (1, HD)],
                               extra_offset=sub * QS + 512),
                           ALU.mult)
                    ctx1.__exit__(None, None, None)
                    if pr + 1 < npair:
                        qkv_next = emit_qkv(pr + 1)
                    # d-tree: in-place halving on [p, (blk*64), d]
                    t1eng = nc.gpsimd if (pr % 10) < int(gp_t1_frac * 10 + 1e-6) \
                        else nc.vector
                    ctx2 = at(g + 0.05); ctx2.__enter__()
                    u1v = u1.rearrange("p (a d) -> p a d", d=HD)
                    w = HD
                    while w > 2:
                        eng = t1eng
                        if (w == HD and u1eng is nc.gpsimd
                                and (pr % 10) < int(gp_t1l1_frac * 10 + 1e-6)):
                            eng = nc.gpsimd   # L1 rides GP right after u1m
                        tt(eng, u1v[:, :, 0:w // 2], u1v[:, :, 0:w // 2],
                           u1v[:, :, w // 2:w], ALU.add)
                        w //= 2
                    s_l = spool.tile([128, 2 * NH * NH], BF16, tag="s")
                    nc.vector.tensor_tensor(
                        out=s_l.rearrange("p (a u) -> p a u", u=1),
                        in0=u1v[:, :, 0:1], in1=u1v[:, :, 1:2], op=ALU.add)
                    # softmax over g: E = exp(S/8); logits O(1) so no max-sub
                    # (high priority: DVE's d-sum stalls behind ACT's bulk
                    # copies otherwise -- strict per-engine FIFO)
                    ctx2.__exit__(None, None, None)
                    if KDBG == 5:
                        of = outsb.tile([128, 2 * C], F32, tag="out", name="ot")
                        nc.vector.tensor_copy(out=of[:, 0:128], in_=s_l)
                        nc.sync.dma_start(
                            out=out_d[b, 0:128, blk0 * 128:blk0 * 128 + 128],
                            in_=of[:, 0:128])
                        continue
                    e_l = spool.tile([128, 2 * NH * NH], BF16, tag="e")
                    with at(g + 0.38):
                        nc.scalar.activation(out=e_l, in_=s_l, func=ACTF.Exp,
                                             scale=0.125)
                    # deferred AV of the previous pair fills the exp-latency
                    # window on DVE (software pipeline, depth 2); tree2 of the
                    # pair before it is deferred one more slot
                    t2c = pending_av() if pending_av is not None else None
                    if pending_t2 is not None:
                        pending_t2()
                    pending_t2 = t2c
                    ctx3 = at(g + 0.42); ctx3.__enter__()
                    d_l = spool.tile([128, 2 * NH], F32, tag="d")
                    nc.vector.tensor_reduce(
                        out=d_l, in_=e_l.rearrange("p (h g) -> p h g", g=NH),
                        axis=AX.X, op=ALU.add)
                    r_l = spool.tile([128, 2 * NH], F32, tag="r")
                    nc.vector.reciprocal(out=r_l, in_=d_l)
                    a_l = spool.tile([128, 2 * NH * NH], BF16, tag="a")
                    aleng = nc.gpsimd if gp_al else nc.vector
                    tt(aleng, a_l.rearrange("p (h g) -> p h g", g=NH),
                       e_l.rearrange("p (h g) -> p h g", g=NH),
                       _bc(r_l, [(1, 2 * NH), (0, NH)]),
                       ALU.mult)
                    # ---- AV: u2[(blk,h,d,g)] = A[blk,h,g] * V'[blk,d,g]
                    # (V columns host-permuted to [d*8+g]: unit-stride reads)
                    ctx3.__exit__(None, None, None)

                    if KDBG == 3:
                        of = outsb.tile([128, 2 * C], F32, tag="out", name="ot")
                        nc.vector.tensor_copy(out=of[:, 0:128],
                                              in_=_bc(a_l, [(1, 128)]))
                        nc.sync.dma_start(
                            out=out_d[b, 0:128, blk0 * 128:blk0 * 128 + 128],
                            in_=of[:, 0:128])
                        continue

                    def make_av(a_l, qkv, blk0, pr, g):
                        def av():
                            ctx4 = at(g + 1.06)
                            ctx4.__enter__()
                            u2 = upool.tile([128, 2 * NH * HD * NH], BF16,
                                            tag="u")
                            u2eng = nc.gpsimd \
                                if (pr % 10) < int(gp_u2_frac * 10 + 1e-6) \
                                or (b == nb - 1 and pr >= npair - tail_u2) \
                                else nc.vector
                            for sub in range(2):
                                tt(u2eng,
                                   _bc(u2, [(HD * NH, NH), (NH, HD), (1, NH)],
                                       extra_offset=sub * NH * HD * NH),
                                   _bc(a_l, [(NH, NH), (0, HD), (1, NH)],
                                       extra_offset=sub * NH * NH),
                                   _bc(qkv, [(0, NH), (NH, HD), (1, NH)],
                                       extra_offset=sub * QS + 1024),
                                   ALU.mult)
                            ctx4.__exit__(None, None, None)

                            def t2():
                                nonlocal pending, otc
                                ctx5 = at(g + 2.06)
                                ctx5.__enter__()
                                # g-tree: O = sum_g (deferred one more slot so
                                # a GP assignment never blocks the next u1m)
                                f2 = gp_t2_frac if gp_t2_frac is not None \
                                    else (1.0 if gp_t2 else 0.0)
                                t2eng = nc.gpsimd \
                                    if (pr % 10) < int(f2 * 10 + 1e-6) \
                                    or (b == nb - 1 and pr >= npair - tail_t2) \
                                    else nc.vector
                                uv = u2.rearrange("p (a g) -> p a g", g=NH)
                                w = NH
                                while w > 2:
                                    tt(t2eng, uv[:, :, 0:w // 2],
                                       uv[:, :, 0:w // 2],
                                       uv[:, :, w // 2:w], ALU.add)
                                    w //= 2
                                o_l = spool.tile([128, 2 * C], BF16, tag="o")
                                tt(t2eng,
                                   o_l.rearrange("p (a u) -> p a u", u=1),
                                   uv[:, :, 0:1], uv[:, :, 1:2], ALU.add)
                                ctx5.__exit__(None, None, None)
                                if pending is not None:
                                    pending()
                                pending = make_stage2(o_l, blk0, pr, g)
                            return t2
                        return av
                    # ---- stage 2 (transpose + evict + proj), deferred one
                    # pair so ACT/PE FIFOs aren't blocked by waits on the
                    # GPSIMD tree of the current pair
                    def make_stage2(o_l, blk0, pr, g):
                        def stage2():
                            nonlocal otc
                            if blk0 % bpc == 0:
                                otc = otr.tile([128, CT * cs], BF16, tag="otc",
                                               name="otc")
                            for sub in range(2):
                                blk = blk0 + sub
                                with at(g + 2.02 + sub * 0.08):
                                    pt = pmm.tile([128, 512], BF16, tag="pt")
                                    for ob in range(CT):
                                        nc.tensor.transpose(
                                            pt[:, ob * 128:(ob + 1) * 128],
                                            o_l[:, sub * C + ob * 128:
                                                sub * C + (ob + 1) * 128],
                                            ident)
                                    nc.scalar.copy(
                                        out=_bc(otc, [(cs, CT), (1, 128)],
                                                extra_offset=(blk % bpc) * 128),
                                        in_=_bc(pt, [(128, CT), (1, 128)]))
                            if (blk0 + 2) % bpc == 0:
                                nonlocal pending_out
                                j = blk0 // bpc
                                ncs = slice(j * cs, (j + 1) * cs)
                                pys = []
                                for c in range(CT):
                                    with at(g + 2.55 + c * 0.06):
                                        xr = xbfp.tile([128, cs], BF16, tag="xr",
                                                       name="xr")
                                        nc.sync.dma_start(
                                            out=xr,
                                            in_=xbf_d[b, c * 128:(c + 1) * 128,
                                                      ncs])
                                        py = pmm.tile([128, cs], F32, tag="py",
                                                      name="py")
                                        for ob in range(CT):
                                            nc.tensor.matmul(
                                                py,
                                                pwT[ob][:, c * 128:(c + 1) * 128],
                                                otc[:, ob * cs:(ob + 1) * cs],
                                                start=(ob == 0), stop=False)
                                        nc.tensor.matmul(py, ident, xr,
                                                         start=False, stop=True)
                                    pys.append(py)

                                def make_out(pys, ncs, g):
                                    def out_flush():
                                        for c in range(CT):
                                            with at(g + 3.35 + c * 0.06):
                                                ot = outsb.tile(
                                                    [128, cs], F32,
                                                    tag="out", name="ot")
                                                nc.scalar.activation(
                                                    out=ot, in_=pys[c],
                                                    func=ACTF.Identity,
                                                    bias=pbt[c], scale=1.0)
                                                nc.sync.dma_start(
                                                    out=out_d[b,
                                                              c * 128:(c + 1) * 128,
                                                              ncs],
                                                    in_=ot)
                                    return out_flush

                                if pending_out is not None:
                                    pending_out()
                                pending_out = make_out(pys, ncs, g)
                        return stage2

                    pending_av = make_av(a_l, qkv, blk0, pr, g)
                t2c = pending_av() if pending_av is not None else None
                pending_av = None
                if pending_t2 is not None:
                    pending_t2()
                if t2c is not None:
                    t2c()
                pending_t2 = None
                if pending is not None:
                    pending()
                    pending = None
                if pending_out is not None:
                    pending_out()
                    pending_out = None
    return nc


_CACHE = {}


def host_inputs(x, norm_w, norm_b, qkv_w, qkv_b, proj_w, proj_b):
    """Host-side preprocessing -> the kernel's shared input tensors."""
    bf = ml_dtypes.bfloat16
    # V-part column permutation: store V as [d*8+g] so the AV multiply reads
    # both operands at unit stride (DVE 2x mode).
    vperm = np.arange(3 * C)
    g_i, d_i = np.meshgrid(np.arange(NH), np.arange(HD), indexing="ij")
    vperm[2 * C:] = 2 * C + (d_i * NH + g_i).reshape(-1)   # old[g*64+d] -> new
    inv = np.empty_like(vperm)
    inv[vperm] = np.arange(3 * C)
    wq_p = qkv_w[inv]        # new column j holds old channel inv[j]
    qkvb_p = np.ascontiguousarray(qkv_b[inv])
    wqkvT = np.ascontiguousarray(wq_p.T).astype(bf)           # [C, 3C]
    pwT = np.ascontiguousarray(proj_w.T).astype(bf)           # [C(o), C(c)]
    ident = np.eye(128, dtype=np.float32).astype(bf)
    # group indicator: ind[c, g] = 1 if channel c (tile-local) in group g
    ind = np.zeros((128, 8), dtype=np.float32)
    for c in range(128):
        ind[c, c // GSIZE] = 1.0
    indT = np.ascontiguousarray(ind.T)
    return dict(wqkvT=wqkvT, pwT=pwT,
                normw=np.asarray(norm_w, np.float32),
                normb=np.asarray(norm_b, np.float32),
                qkvb=qkvb_p, pbeff=np.asarray(proj_b, np.float32),
                ident=ident, ind=ind.astype(bf), indT=indT.astype(bf))


def kernel(x, norm_w, norm_b, qkv_w, qkv_b, proj_w, proj_b):
    x = np.asarray(x, np.float32)
    norm_w = np.asarray(norm_w, np.float32)
    norm_b = np.asarray(norm_b, np.float32)
    qkv_w = np.asarray(qkv_w, np.float32)
    qkv_b = np.asarray(qkv_b, np.float32)
    proj_w = np.asarray(proj_w, np.float32)
    proj_b = np.asarray(proj_b, np.float32)

    qk_bias = bool(np.any(qkv_b != 0))
    key = ("full", qk_bias)
    if key not in _CACHE:
        nc_new = build_kernel(qk_bias=qk_bias)
        _cap_sync_waits(nc_new)   # HW path only; CoreSim rejects bare NoOps
        _CACHE[key] = nc_new
    nc = _CACHE[key]

    shared = host_inputs(x, norm_w, norm_b, qkv_w, qkv_b, proj_w, proj_b)
    xs = x.reshape(B, C, N)
    xbf = xs.astype(ml_dtypes.bfloat16)
    in_maps = [dict(x=np.ascontiguousarray(xs[c * NB:(c + 1) * NB]),
                    xbf=np.ascontiguousarray(xbf[c * NB:(c + 1) * NB]),
                    **shared)
               for c in range(NCORES)]
    res = run_bass_kernel_spmd(nc, in_maps, core_ids=list(range(NCORES)),
                               trace=bool(os.environ.get("KERNEL_TRACE")))
    global LAST_RES
    LAST_RES = res
    out = np.concatenate([res.results[c]["out"] for c in range(NCORES)], axis=0)
    return out.reshape(B, C, HH, WW).astype(np.float32)


LAST_RES = None


# revision 66
# speedup vs baseline: 1.2164x; 1.0061x over previous
"""Trainium2 Bass kernel for nn_AttentionBlock (GroupNorm + per-position
head-axis attention + proj + residual).

Sharding: data-parallel over batch B=16 -> 2 batches per core x 8 cores.

Per-core pipeline (v2):
  1. GroupNorm(32): x streamed twice in half-row chunks (stats pass +
     apply pass) so it never needs full SBUF residency; cross-partition
     group aggregation + broadcast via two tiny PE indicator matmuls
     (replaces v1's slow SBUF->SBUF DMA gathers); per-group scalar math
     batched across the 4 channel tiles in single small DVE ops. The
     next batch's norm is emitted mid-way through the current batch so
     it fully overlaps attention.
  2. QKV matmul n-major (positions on PSUM partitions); PSUM evicted by
     one ACT copy per block into a [128, 2-block, 3C] bf16 tile.
  3. Attention in PAIRS of 128-position blocks so the ~220ns DVE fixed
     cost per instruction is paid half as often wherever APs allow
     (walrus caps compute APs at 3 free dims, so the q*k and A*V
     broadcast-multiplies are per-block): d-reduction and g-reduction
     as in-place halving add-trees (DVE 2x bf16); softmax with exp on
     ACT (1/8 scale folded, no max-subtraction -- logits are O(1) by
     construction). Software pipelining: GPSIMD runs the u1=q*k multiply
     one pair ahead (one of the two blocks goes to DVE on every 2nd
     pair to balance makespans); the AV multiply of pair p-1 and the
     g-tree of pair p-2 run between tree1(p) and softmax(p) on DVE,
     hiding the ACT exp latency entirely.
  4. O transposed to C-major via PE transposes (evicted by one strided
     ACT copy per block); proj matmul accumulates the residual via an
     extra identity-weight matmul against a host-provided bf16 copy of
     x; PSUM evicted on ACT with the proj bias folded in; DMA out. Both
     stages run 1-2 pairs behind the compute so waits never block the
     ACT/PE FIFOs.

Host-side preprocessing: weight transposes + bf16 casts + V-column
permute to [d*8+g] (AV multiply reads both operands unit-stride so DVE
2x mode applies) + bf16 x copy (residual matmul operand) + group
indicator matrices for the norm matmuls.

_cap_sync_waits: this walrus build accepts only ONE sync wait per compute
instruction; Tile emits more. The pass hoists excess waits onto
same-engine InstNoOps inserted immediately before the offender.
"""

import contextlib
import os

import numpy as np
import ml_dtypes

import concourse.bass as bass
import concourse.mybir as mybir
import concourse.tile as tile
from concourse.bass_utils import run_bass_kernel_spmd

F32 = mybir.dt.float32
BF16 = mybir.dt.bfloat16

B, C, HH, WW = 16, 512, 64, 64
N = HH * WW            # 4096
NB = 2                 # batches per core
NCORES = 8
NH, HD = 8, 64         # heads, head dim
GROUPS = 32
GSIZE = C // GROUPS    # 16 channels per group
EPS = 1e-5
CT = C // 128          # 4 channel tiles
NBLK = N // 128        # 32 position blocks per batch

AX = mybir.AxisListType
ALU = mybir.AluOpType
ACTF = mybir.ActivationFunctionType


def _bc(t, dims, extra_offset=0):
    """AP over tile/AP `t` with explicit free [step,count] dims (elem units)."""
    return bass.AP(tensor=t.tensor, offset=t.offset + extra_offset,
                   ap=[list(t.ap[0])] + [list(d) for d in dims])


def _cap_sync_waits(nc):
    """Walrus allows at most 1 sync wait per compute instruction; Tile can
    emit more. Hoist the excess onto same-engine InstNoOps inserted
    immediately before the offender."""
    import bass_rust
    n = 0
    for f in nc.m.functions:
        for blk in f.blocks:
            il = blk.instructions
            i = 0
            while i < len(il):
                ins = il[i]
                si = getattr(ins, "sync_info", None)
                if si is not None and si.on_wait and len(si.on_wait) > 1:
                    waits = list(si.on_wait)
                    for w in waits[:-1]:
                        nop = mybir.InstNoOp(name=f"W-abs-{n}", ins=[], outs=[])
                        n += 1
                        nop.engine = ins.engine
                        nop.sync_info = bass_rust.SyncInfo(on_wait=[w],
                                                           on_update=[])
                        il.insert(i, nop)
                        i += 1
                    si.on_wait = waits[-1:]
                i += 1
    return n


KDBG = int(os.environ.get("KDBG", "0"))


def build_kernel(nb=NB, nblk=NBLK, qk_bias=False, gp_u2_frac=0.0,
                 gp_t2=False, gp_t1_frac=0.0, gp_u1_frac=1.0,
                 period_ms=0.0, pq_bufs=2, pmm_bufs=1, qkv_bufs=3,
                 spool_bufs=2, gp_t2_frac=None, gp_stats=False,
                 gp_al=False, gp_t1l1_frac=0.0, tail_t2=0, tail_u2=0,
                 u1_dve_sub_every=2):
    n = nblk * 128
    npair = max(1, nblk // 2)      # block pairs (256 positions each)
    cs = min(512, n)               # proj/residual n-chunk
    nc = bass.Bass()

    x_d = nc.dram_tensor("x", [nb, C, n], F32, kind="ExternalInput")
    xbf_d = nc.dram_tensor("xbf", [nb, C, n], BF16, kind="ExternalInput")
    wqkvT_d = nc.dram_tensor("wqkvT", [C, 3 * C], BF16, kind="ExternalInput")
    pwT_d = nc.dram_tensor("pwT", [C, C], BF16, kind="ExternalInput")
    normw_d = nc.dram_tensor("normw", [C], F32, kind="ExternalInput")
    normb_d = nc.dram_tensor("normb", [C], F32, kind="ExternalInput")
    qkvb_d = nc.dram_tensor("qkvb", [3 * C], F32, kind="ExternalInput")
    pbeff_d = nc.dram_tensor("pbeff", [C], F32, kind="ExternalInput")
    ident_d = nc.dram_tensor("ident", [128, 128], BF16, kind="ExternalInput")
    ind_d = nc.dram_tensor("ind", [128, 8], BF16, kind="ExternalInput")
    indT_d = nc.dram_tensor("indT", [8, 128], BF16, kind="ExternalInput")
    out_d = nc.dram_tensor("out", [nb, C, n], F32, kind="ExternalOutput")

    with tile.TileContext(nc) as tc:
        def tt(eng, out, in0, in1, op):
            # (gpsimd scalar_tensor_tensor is priced better by the cost model
            # but does not compile for the Pool engine -- plain TT only)
            eng.tensor_tensor(out=out, in0=in0, in1=in1, op=op)

        def at(slot):
            """Virtual-clock stamp (scheduling-only; TimelineSim/HW replay is
            semaphore-timed). Shapes each engine's static FIFO order so ops
            that wait long never sit ahead of ops that are ready."""
            if period_ms <= 0:
                return contextlib.nullcontext()
            return tc.tile_wait_until(max(0.0, slot) * period_ms)

        with (
            tc.tile_pool(name="consts", bufs=1) as consts,
            tc.tile_pool(name="xpool", bufs=4) as xpool,
            tc.tile_pool(name="hlo", bufs=2) as hlo,
            tc.tile_pool(name="hhi", bufs=1) as hhi,
            tc.tile_pool(name="xbfp", bufs=2) as xbfp,
            tc.tile_pool(name="otr", bufs=2) as otr,
            tc.tile_pool(name="stats", bufs=2) as stats,
            tc.tile_pool(name="scb", bufs=2) as scb,
            tc.tile_pool(name="qkvsb", bufs=qkv_bufs) as qkvsb,
            tc.tile_pool(name="upool", bufs=4) as upool,
            tc.tile_pool(name="spool", bufs=spool_bufs) as spool,
            tc.tile_pool(name="outsb", bufs=3) as outsb,
            tc.tile_pool(name="pqkv", bufs=pq_bufs, space="PSUM") as pqkv,
            tc.tile_pool(name="pmm", bufs=pmm_bufs, space="PSUM") as pmm,
        ):
            # ---- constants / weights in SBUF ----
            wqkvT = []
            for c in range(CT):
                t = consts.tile([128, 3 * C], BF16, tag=f"wq{c}")
                nc.sync.dma_start(out=t, in_=wqkvT_d[c * 128:(c + 1) * 128, :])
                wqkvT.append(t)
            pwT = []
            for o in range(CT):
                t = consts.tile([128, C], BF16, tag=f"pw{o}")
                nc.sync.dma_start(out=t, in_=pwT_d[o * 128:(o + 1) * 128, :])
                pwT.append(t)
            ident = consts.tile([128, 128], BF16, tag="ident")
            nc.sync.dma_start(out=ident, in_=ident_d[:, :])
            ind = consts.tile([128, 8], BF16, tag="ind")
            nc.sync.dma_start(out=ind, in_=ind_d[:, :])
            indT = consts.tile([8, 128], BF16, tag="indT")
            nc.sync.dma_start(out=indT, in_=indT_d[:, :])
            nwt, nbt, pbt = [], [], []
            for c in range(CT):
                sl = slice(c * 128, (c + 1) * 128)
                t1 = consts.tile([128, 1], F32, tag=f"nw{c}")
                nc.sync.dma_start(out=t1, in_=normw_d[sl].rearrange("(p u) -> p u", u=1))
                nwt.append(t1)
                t2 = consts.tile([128, 1], F32, tag=f"nb{c}")
                nc.sync.dma_start(out=t2, in_=normb_d[sl].rearrange("(p u) -> p u", u=1))
                nbt.append(t2)
                t3 = consts.tile([128, 1], F32, tag=f"pb{c}")
                nc.sync.dma_start(out=t3, in_=pbeff_d[sl].rearrange("(p u) -> p u", u=1))
                pbt.append(t3)
            epst = consts.tile([8, 1], F32, tag="eps")
            nc.vector.memset(epst, 256.0 * EPS)
            qkbias = None
            if qk_bias:
                qkbias = consts.tile([128, 3 * C], F32, tag="qkb")
                nc.sync.dma_start(
                    out=qkbias,
                    in_=bass.AP(tensor=qkvb_d.ap().tensor, offset=0,
                                ap=[[0, 128], [1, 3 * C]]))

            def emit_norm(b, g0):
                """GroupNorm scales/biases + normalized h for batch b.

                x is streamed twice in [128, n/2] chunks (stats pass, then
                apply pass) so it never needs full SBUF residency."""
                nh2 = n // 2
                nsub = max(1, n // 512)
                # --- stats pass: stream x chunks, bn_stats into st[c] ---
                st_c = []
                sd = nc.vector.BN_STATS_DIM
                for c in range(CT):
                    st = stats.tile([128, nsub, sd], F32, tag=f"bnst{c}",
                                    name=f"bnst{c}")
                    st_c.append(st)
                for c in range(CT):
                    for half in range(2):
                        hs = slice(half * nh2, (half + 1) * nh2)
                        with at(g0 - 8 + (c * 2 + half) * 0.5):
                            t = xpool.tile([128, nh2], F32, tag="xs", name="xs")
                            nc.sync.dma_start(
                                out=t, in_=x_d[b, c * 128:(c + 1) * 128, hs])
                            xv = t.rearrange("p (s f) -> p s f", s=nsub // 2)
                            seng = nc.gpsimd if gp_stats else nc.vector
                            for s in range(nsub // 2):
                                seng.bn_stats(
                                    out=st_c[c][:, half * (nsub // 2) + s, :],
                                    in_=xv[:, s, :])
                # per-channel running stats -> st2all [128, (ct,2)] bf16
                ctx_norm = at(g0 - 4)
                ctx_norm.__enter__()
                st2all = stats.tile([128, 2 * CT], BF16, tag="st2all",
                                    name="st2all")
                for c in range(CT):
                    mv = stats.tile([128, nc.vector.BN_AGGR_DIM], F32, tag="bnmv")
                    nc.vector.bn_aggr(out=mv, in_=st_c[c])
                    # col0 = mean, col1 = E[x^2] = var + mean^2
                    nc.vector.tensor_copy(out=st2all[:, 2 * c:2 * c + 1],
                                          in_=mv[:, 0:1])
                    nc.vector.scalar_tensor_tensor(
                        out=st2all[:, 2 * c + 1:2 * c + 2], in0=mv[:, 0:1],
                        scalar=mv[:, 0:1], in1=mv[:, 1:2],
                        op0=ALU.mult, op1=ALU.add)
                if KDBG == 2:
                    sc_t, bi_t = [], []
                    for c in range(CT):
                        sc = scb.tile([128, 1], F32, tag=f"sc{c}", name=f"sc{c}")
                        bi = scb.tile([128, 1], F32, tag=f"bi{c}", name=f"bi{c}")
                        nc.vector.memset(sc, 1.0)
                        nc.vector.memset(bi, 0.0)
                        sc_t.append(sc)
                        bi_t.append(bi)
                    ctx_norm.__exit__(None, None, None)
                    ht = [[], []]
                    for half in range(2):
                        hp = hlo if half == 0 else hhi
                        hs = slice(half * nh2, (half + 1) * nh2)
                        for c in range(CT):
                            with at(g0 - 2.5 + (half * CT + c) * 0.25):
                                xa = xpool.tile([128, nh2], F32, tag="xs",
                                                name="xa")
                                nc.sync.dma_start(
                                    out=xa,
                                    in_=x_d[b, c * 128:(c + 1) * 128, hs])
                                t = hp.tile([128, nh2], BF16,
                                            tag=f"h{half}_{c}",
                                            name=f"h{half}_{c}")
                                nc.scalar.activation(out=t, in_=xa,
                                                     func=ACTF.Identity,
                                                     bias=bi_t[c],
                                                     scale=sc_t[c])
                            ht[half].append(t)
                    return ht
                # group aggregation: psum [8, (ct,2)] = sum over 16 channels
                # (full-size py-shaped tile so the PSUM tag stays uniform)
                pg_t = pmm.tile([128, 512], F32, tag="py", name="pg")
                pg = pg_t[0:8, 0:2 * CT]
                nc.tensor.matmul(pg, ind, st2all, start=True, stop=True)
                s8 = stats.tile([8, 2 * CT], F32, tag="s8", name="s8")
                nc.vector.tensor_copy(out=s8, in_=pg)
                # group math on 8 partitions, batched over ct via strided APs
                sm = _bc(s8, [(2, CT)])                  # sum of means
                se = _bc(s8, [(2, CT)], extra_offset=1)  # sum of E[x^2]
                m2 = stats.tile([8, CT], F32, tag="m2", name="m2")
                nc.vector.tensor_mul(m2, sm, sm)
                v256 = stats.tile([8, CT], F32, tag="v256", name="v256")
                nc.vector.scalar_tensor_tensor(
                    out=v256, in0=se, scalar=16.0, in1=m2,
                    op0=ALU.mult, op1=ALU.subtract)
                # rstd/16 = 1/sqrt(256(var+eps))
                sg = stats.tile([8, CT], F32, tag="sg", name="sg")
                nc.scalar.activation(out=sg, in_=v256, func=ACTF.Sqrt,
                                     scale=1.0, bias=epst)
                rg = stats.tile([8, CT], F32, tag="rg", name="rg")
                nc.vector.reciprocal(out=rg, in_=sg)
                # broadcast payload [8, (ct,2)] bf16: col0=rstd, col1=mean
                b8 = stats.tile([8, 2 * CT], BF16, tag="b8", name="b8")
                nc.vector.tensor_scalar(
                    out=_bc(b8, [(2, CT)]), in0=rg, scalar1=16.0,
                    scalar2=None, op0=ALU.mult)
                nc.vector.tensor_scalar(
                    out=_bc(b8, [(2, CT)], extra_offset=1), in0=sm,
                    scalar1=1.0 / 16.0, scalar2=None, op0=ALU.mult)
                pb_t = pmm.tile([128, 512], F32, tag="py", name="pb128")
                pb128 = pb_t[:, 0:2 * CT]
                nc.tensor.matmul(pb128, indT, b8, start=True, stop=True)
                c2 = scb.tile([128, 2 * CT], F32, tag="c2", name="c2")
                nc.vector.tensor_copy(out=c2, in_=pb128)
                # sc[ct] = rstd * nw ; bi[ct] = nb - mean*sc
                sc_t, bi_t = [], []
                for c in range(CT):
                    sc = scb.tile([128, 1], F32, tag=f"sc{c}", name=f"sc{c}")
                    bi = scb.tile([128, 1], F32, tag=f"bi{c}", name=f"bi{c}")
                    nc.vector.tensor_mul(sc, c2[:, 2 * c:2 * c + 1], nwt[c])
                    tmp = stats.tile([128, 1], F32, tag="tmp")
                    nc.vector.tensor_mul(tmp, c2[:, 2 * c + 1:2 * c + 2], sc)
                    nc.vector.tensor_sub(bi, nbt[c], tmp)
                    sc_t.append(sc)
                    bi_t.append(bi)
                ctx_norm.__exit__(None, None, None)
                # --- apply pass: re-stream x chunks -> normalized h ---
                ht = [[], []]
                for half in range(2):
                    hp = hlo if half == 0 else hhi
                    hs = slice(half * nh2, (half + 1) * nh2)
                    for c in range(CT):
                        with at(g0 - 2.5 + (half * CT + c) * 0.25):
                            xa = xpool.tile([128, nh2], F32, tag="xs", name="xa")
                            nc.sync.dma_start(
                                out=xa, in_=x_d[b, c * 128:(c + 1) * 128, hs])
                            t = hp.tile([128, nh2], BF16, tag=f"h{half}_{c}",
                                        name=f"h{half}_{c}")
                            nc.scalar.activation(out=t, in_=xa,
                                                 func=ACTF.Identity,
                                                 bias=bi_t[c], scale=sc_t[c])
                        ht[half].append(t)
                return ht

            ht_next = None
            for b in range(nb):
                if ht_next is None:
                    ht_next = emit_norm(b, b * npair)
                ht = ht_next
                ht_next = None
                bpc = cs // 128                 # blocks per out-chunk (4)
                otc = None
                pending = None
                pending_av = None
                pending_t2 = None
                pending_out = None
                def emit_qkv(pr):
                    g = b * npair + pr
                    blk0 = 2 * pr
                    qkv = qkvsb.tile([128, 2, 3 * C], BF16, tag="qkv")
                    nh2 = n // 2
                    hb = max(1, nblk // 2)
                    for sub in range(2):
                        blk = blk0 + sub
                        half = min(blk // hb, 1)
                        hslice = slice(blk * 128 - half * nh2,
                                       (blk + 1) * 128 - half * nh2)
                        with at(g - 0.7 + sub * 0.1):
                            p = pqkv.tile([128, 3 * C], F32, tag="pq",
                                          name=f"pq{sub}")
                            for c in range(CT):
                                lhsT = ht[half][c][:, hslice]
                                for oc in range(3):
                                    nc.tensor.matmul(
                                        p[:, oc * 512:(oc + 1) * 512], lhsT,
                                        wqkvT[c][:, oc * 512:(oc + 1) * 512],
                                        start=(c == 0), stop=(c == CT - 1))
                            if qkbias is not None:
                                nc.vector.tensor_add(out=qkv[:, sub, :], in0=p,
                                                     in1=qkbias)
                            else:
                                nc.scalar.copy(out=qkv[:, sub, :], in_=p)
                    return qkv

                qkv_next = emit_qkv(0)
                for pr in range(npair):
                    if pr == npair // 2 and b + 1 < nb:
                        # hoist next batch's GroupNorm into this batch's
                        # midsection so its stats/apply overlap attention
                        ht_next = emit_norm(b + 1, (b + 1) * npair)
                    g = b * npair + pr          # global pair slot
                    blk0 = 2 * pr               # first block of the pair
                    qkv = qkv_next

                    # q/k/v APs: qkv [128, (blk, 3C)]
                    QS = 3 * C
                    # ---- logits: u1[(blk,h,g,d)] = q[blk,h,d] * k[blk,g,d]
                    if KDBG == 1:
                        of = outsb.tile([128, 2 * C], F32, tag="out", name="ot")
                        nc.vector.tensor_copy(
                            out=of,
                            in_=_bc(qkv, [(QS, 2), (1, C)], extra_offset=1024))
                        nc.sync.dma_start(
                            out=out_d[b, 0:128, blk0 * 128:(blk0 + 2) * 128]
                                .rearrange("p (a f) -> p a f", a=2)
                                .rearrange("p a f -> p (a f)"),
                            in_=of[:, 0:256])
                        continue
                    u1 = upool.tile([128, 2 * NH * NH * HD], BF16, tag="u")
                    u1eng = nc.gpsimd if (pr % 10) < int(gp_u1_frac * 10 + 1e-6) \
                        else nc.vector
                    if pr == 0:
                        u1eng = nc.vector   # skip GP latency on the batch's first pair
                    ctx1 = at(g - 0.45); ctx1.__enter__()
                    # per-block 3D APs: the walrus ISA pattern caps compute
                    # APs at 3 free dims and the pair-strided 4D form does
                    # not merge
                    for sub in range(2):
                        eng_s = u1eng
                        if u1_dve_sub_every and sub == 1:
                            k = u1_dve_sub_every
                            hit = (pr % k == 0) if k > 0 else (pr % (-k) != 0)
                            if hit:
                                eng_s = nc.vector
                        tt(eng_s,
                           _bc(u1, [(NH * HD, NH), (HD, NH), (1, HD)],
                               extra_offset=sub * NH * NH * HD),
                           _bc(qkv, [(HD, NH), (0, NH), (1, HD)],
                               extra_offset=sub * QS),
                           _bc(qkv, [(0, NH), (HD, NH), (1, HD)],
                               extra_offset=sub * QS + 512),
                           ALU.mult)
                    ctx1.__exit__(None, None, None)
                    if pr + 1 < npair:
                        qkv_next = emit_qkv(pr + 1)
                    # d-tree: in-place halving on [p, (blk*64), d]
                    t1eng = nc.gpsimd if (pr % 10) < int(gp_t1_frac * 10 + 1e-6) \
                        else nc.vector
                    ctx2 = at(g + 0.05); ctx2.__enter__()
                    u1v = u1.rearrange("p (a d) -> p a d", d=HD)
                    w = HD
                    while w > 2:
                        eng = t1eng
                        if (w == HD and u1eng is nc.gpsimd
                                and (pr % 10) < int(gp_t1l1_frac * 10 + 1e-6)):
                            eng = nc.gpsimd   # L1 rides GP right after u1m
                        tt(eng, u1v[:, :, 0:w // 2], u1v[:, :, 0:w // 2],
                           u1v[:, :, w // 2:w], ALU.add)
                        w //= 2
                    s_l = spool.tile([128, 2 * NH * NH], BF16, tag="s")
                    nc.vector.tensor_tensor(
                        out=s_l.rearrange("p (a u) -> p a u", u=1),
                        in0=u1v[:, :, 0:1], in1=u1v[:, :, 1:2], op=ALU.add)
                    # softmax over g: E = exp(S/8); logits O(1) so no max-sub
                    # (high priority: DVE's d-sum stalls behind ACT's bulk
                    # copies otherwise -- strict per-engine FIFO)
                    ctx2.__exit__(None, None, None)
                    if KDBG == 5:
                        of = outsb.tile([128, 2 * C], F32, tag="out", name="ot")
                        nc.vector.tensor_copy(out=of[:, 0:128], in_=s_l)
                        nc.sync.dma_start(
                            out=out_d[b, 0:128, blk0 * 128:blk0 * 128 + 128],
                            in_=of[:, 0:128])
                        continue
                    e_l = spool.tile([128, 2 * NH * NH], BF16, tag="e")
                    with at(g + 0.38):
                        nc.scalar.activation(out=e_l, in_=s_l, func=ACTF.Exp,
                                             scale=0.125)
                    # deferred AV of the previous pair fills the exp-latency
                    # window on DVE (software pipeline, depth 2); tree2 of the
                    # pair before it is deferred one more slot
                    t2c = pending_av() if pending_av is not None else None
                    if pending_t2 is not None:
                        pending_t2()
                    pending_t2 = t2c
                    ctx3 = at(g + 0.42); ctx3.__enter__()
                    d_l = spool.tile([128, 2 * NH], F32, tag="d")
                    nc.vector.tensor_reduce(
                        out=d_l, in_=e_l.rearrange("p (h g) -> p h g", g=NH),
                        axis=AX.X, op=ALU.add)
                    r_l = spool.tile([128, 2 * NH], F32, tag="r")
                    nc.vector.reciprocal(out=r_l, in_=d_l)
                    a_l = spool.tile([128, 2 * NH * NH], BF16, tag="a")
                    aleng = nc.gpsimd if gp_al else nc.vector
                    tt(aleng, a_l.rearrange("p (h g) -> p h g", g=NH),
                       e_l.rearrange("p (h g) -> p h g", g=NH),
                       _bc(r_l, [(1, 2 * NH), (0, NH)]),
                       ALU.mult)
                    # ---- AV: u2[(blk,h,d,g)] = A[blk,h,g] * V'[blk,d,g]
                    # (V columns host-permuted to [d*8+g]: unit-stride reads)
                    ctx3.__exit__(None, None, None)

                    if KDBG == 3:
                        of = outsb.tile([128, 2 * C], F32, tag="out", name="ot")
                        nc.vector.tensor_copy(out=of[:, 0:128],
                                              in_=_bc(a_l, [(1, 128)]))
                        nc.sync.dma_start(
                            out=out_d[b, 0:128, blk0 * 128:blk0 * 128 + 128],
                            in_=of[:, 0:128])
                        continue

                    def make_av(a_l, qkv, blk0, pr, g):
                        def av():
                            ctx4 = at(g + 1.06)
                            ctx4.__enter__()
                            u2 = upool.tile([128, 2 * NH * HD * NH], BF16,
                                            tag="u")
                            u2eng = nc.gpsimd \
                                if (pr % 10) < int(gp_u2_frac * 10 + 1e-6) \
                                or (b == nb - 1 and pr >= npair - tail_u2) \
                                else nc.vector
                            for sub in range(2):
                                tt(u2eng,
                                   _bc(u2, [(HD * NH, NH), (NH, HD), (1, NH)],
                                       extra_offset=sub * NH * HD * NH),
                                   _bc(a_l, [(NH, NH), (0, HD), (1, NH)],
                                       extra_offset=sub * NH * NH),
                                   _bc(qkv, [(0, NH), (NH, HD), (1, NH)],
                                       extra_offset=sub * QS + 1024),
                                   ALU.mult)
                            ctx4.__exit__(None, None, None)

                            def t2():
                                nonlocal pending, otc
                                ctx5 = at(g + 2.06)
                                ctx5.__enter__()
                                # g-tree: O = sum_g (deferred one more slot so
                                # a GP assignment never blocks the next u1m)
                                f2 = gp_t2_frac if gp_t2_frac is not None \
                                    else (1.0 if gp_t2 else 0.0)
                                t2eng = nc.gpsimd \
                                    if (pr % 10) < int(f2 * 10 + 1e-6) \
                                    or (b == nb - 1 and pr >= npair - tail_t2) \
                                    else nc.vector
                                uv = u2.rearrange("p (a g) -> p a g", g=NH)
                                w = NH
                                while w > 2:
                                    tt(t2eng, uv[:, :, 0:w // 2],
                                       uv[:, :, 0:w // 2],
                                       uv[:, :, w // 2:w], ALU.add)
                                    w //= 2
                                o_l = spool.tile([128, 2 * C], BF16, tag="o")
                                tt(t2eng,
                                   o_l.rearrange("p (a u) -> p a u", u=1),
                                   uv[:, :, 0:1], uv[:, :, 1:2], ALU.add)
                                ctx5.__exit__(None, None, None)
                                if pending is not None:
                                    pending()
                                pending = make_stage2(o_l, blk0, pr, g)
                            return t2
                        return av
                    # ---- stage 2 (transpose + evict + proj), deferred one
                    # pair so ACT/PE FIFOs aren't blocked by waits on the
                    # GPSIMD tree of the current pair
                    def make_stage2(o_l, blk0, pr, g):
                        def stage2():
                            nonlocal otc
                            if blk0 % bpc == 0:
                                otc = otr.tile([128, CT * cs], BF16, tag="otc",
                                               name="otc")
                            for sub in range(2):
                                blk = blk0 + sub
                                with at(g + 2.02 + sub * 0.08):
                                    pt = pmm.tile([128, 512], BF16, tag="pt")
                                    for ob in range(CT):
                                        nc.tensor.transpose(
                                            pt[:, ob * 128:(ob + 1) * 128],
                                            o_l[:, sub * C + ob * 128:
                                                sub * C + (ob + 1) * 128],
                                            ident)
                                    nc.scalar.copy(
                                        out=_bc(otc, [(cs, CT), (1, 128)],
                                                extra_offset=(blk % bpc) * 128),
                                        in_=_bc(pt, [(128, CT), (1, 128)]))
                            if (blk0 + 2) % bpc == 0:
                                nonlocal pending_out
                                j = blk0 // bpc
                                ncs = slice(j * cs, (j + 1) * cs)
                                pys = []
                                for c in range(CT):
                                    with at(g + 2.55 + c * 0.06):
                                        xr = xbfp.tile([128, cs], BF16, tag="xr",
                                                       name="xr")
                                        nc.sync.dma_start(
                                            out=xr,
                                            in_=xbf_d[b, c * 128:(c + 1) * 128,
                                                      ncs])
                                        py = pmm.tile([128, cs], F32, tag="py",
                                                      name="py")
                                        for ob in range(CT):
                                            nc.tensor.matmul(
                                                py,
                                                pwT[ob][:, c * 128:(c + 1) * 128],
                                                otc[:, ob * cs:(ob + 1) * cs],
                                                start=(ob == 0), stop=False)
                                        nc.tensor.matmul(py, ident, xr,
                                                         start=False, stop=True)
                                    pys.append(py)

                                def make_out(pys, ncs, g):
                                    def out_flush():
                                        for c in range(CT):
                                            with at(g + 3.35 + c * 0.06):
                                                ot = outsb.tile(
                                                    [128, cs], F32,
                                                    tag="out", name="ot")
                                                nc.scalar.activation(
                                                    out=ot, in_=pys[c],
                                                    func=ACTF.Identity,
                                                    bias=pbt[c], scale=1.0)
                                                nc.sync.dma_start(
                                                    out=out_d[b,
                                                              c * 128:(c + 1) * 128,
                                                              ncs],
                                                    in_=ot)
                                    return out_flush

                                if pending_out is not None:
                                    pending_out()
                                pending_out = make_out(pys, ncs, g)
                        return stage2

                    pending_av = make_av(a_l, qkv, blk0, pr, g)
                t2c = pending_av() if pending_av is not None else None
                pending_av = None
                if pending_t2 is not None:
                    pending_t2()
                if t2c is not None:
                    t2c()
                pending_t2 = None
                if pending is not None:
                    pending()
                    pending = None
                if pending_out is not None:
                    pending_out()
                    pending_out = None
    return nc


_CACHE = {}


def host_inputs(x, norm_w, norm_b, qkv_w, qkv_b, proj_w, proj_b):
    """Host-side preprocessing -> the kernel's shared input tensors."""
    bf = ml_dtypes.bfloat16
    # V-part column permutation: store V as [d*8+g] so the AV multiply reads
    # both operands at unit stride (DVE 2x mode).
    vperm = np.arange(3 * C)
    g_i, d_i = np.meshgrid(np.arange(NH), np.arange(HD), indexing="ij")
    vperm[2 * C:] = 2 * C + (d_i * NH + g_i).reshape(-1)   # old[g*64+d] -> new
    inv = np.empty_like(vperm)
    inv[vperm] = np.arange(3 * C)
    wq_p = qkv_w[inv]        # new column j holds old channel inv[j]
    qkvb_p = np.ascontiguousarray(qkv_b[inv])
    wqkvT = np.ascontiguousarray(wq_p.T).astype(bf)           # [C, 3C]
    pwT = np.ascontiguousarray(proj_w.T).astype(bf)           # [C(o), C(c)]
    ident = np.eye(128, dtype=np.float32).astype(bf)
    # group indicator: ind[c, g] = 1 if channel c (tile-local) in group g
    ind = np.zeros((128, 8), dtype=np.float32)
    for c in range(128):
        ind[c, c // GSIZE] = 1.0
    indT = np.ascontiguousarray(ind.T)
    return dict(wqkvT=wqkvT, pwT=pwT,
                normw=np.asarray(norm_w, np.float32),
                normb=np.asarray(norm_b, np.float32),
                qkvb=qkvb_p, pbeff=np.asarray(proj_b, np.float32),
                ident=ident, ind=ind.astype(bf), indT=indT.astype(bf))


def kernel(x, norm_w, norm_b, qkv_w, qkv_b, proj_w, proj_b):
    x = np.asarray(x, np.float32)
    norm_w = np.asarray(norm_w, np.float32)
    norm_b = np.asarray(norm_b, np.float32)
    qkv_w = np.asarray(qkv_w, np.float32)
    qkv_b = np.asarray(qkv_b, np.float32)
    proj_w = np.asarray(proj_w, np.float32)
    proj_b = np.asarray(proj_b, np.float32)

    qk_bias = bool(np.any(qkv_b != 0))
    key = ("full", qk_bias)
    if key not in _CACHE:
        nc_new = build_kernel(qk_bias=qk_bias)
        _cap_sync_waits(nc_new)   # HW path only; CoreSim rejects bare NoOps
        _CACHE[key] = nc_new
    nc = _CACHE[key]

    shared = host_inputs(x, norm_w, norm_b, qkv_w, qkv_b, proj_w, proj_b)
    xs = x.reshape(B, C, N)
    xbf = xs.astype(ml_dtypes.bfloat16)
    in_maps = [dict(x=np.ascontiguousarray(xs[c * NB:(c + 1) * NB]),
                    xbf=np.ascontiguousarray(xbf[c * NB:(c + 1) * NB]),
                    **shared)
               for c in range(NCORES)]
    res = run_bass_kernel_spmd(nc, in_maps, core_ids=list(range(NCORES)),
                               trace=bool(os.environ.get("KERNEL_TRACE")))
    global LAST_RES
    LAST_RES = res
    out = np.concatenate([res.results[c]["out"] for c in range(NCORES)], axis=0)
    return out.reshape(B, C, HH, WW).astype(np.float32)


LAST_RES = None


# revision 68
# speedup vs baseline: 1.3018x; 1.0702x over previous
"""Trainium2 Bass kernel for nn_AttentionBlock (GroupNorm + per-position
head-axis attention + proj + residual).

Sharding: data-parallel over batch B=16 -> 2 batches per core x 8 cores.

Per-core pipeline (v2):
  1. GroupNorm(32): x streamed twice in half-row chunks (stats pass +
     apply pass) so it never needs full SBUF residency; cross-partition
     group aggregation + broadcast via two tiny PE indicator matmuls
     (replaces v1's slow SBUF->SBUF DMA gathers); per-group scalar math
     batched across the 4 channel tiles in single small DVE ops. The
     next batch's norm is emitted mid-way through the current batch so
     it fully overlaps attention.
  2. QKV matmul n-major (positions on PSUM partitions); PSUM evicted by
     one ACT copy per block into a [128, 2-block, 3C] bf16 tile.
  3. Attention in PAIRS of 128-position blocks so the ~220ns DVE fixed
     cost per instruction is paid half as often wherever APs allow
     (walrus caps compute APs at 3 free dims, so the q*k and A*V
     broadcast-multiplies are per-block): d-reduction and g-reduction
     as in-place halving add-trees (DVE 2x bf16); softmax with exp on
     ACT (1/8 scale folded, no max-subtraction -- logits are O(1) by
     construction). Software pipelining: GPSIMD runs the u1=q*k multiply
     one pair ahead (one of the two blocks goes to DVE on every 2nd
     pair to balance makespans); the AV multiply of pair p-1 and the
     g-tree of pair p-2 run between tree1(p) and softmax(p) on DVE,
     hiding the ACT exp latency entirely.
  4. O transposed to C-major via PE transposes (evicted by one strided
     ACT copy per block); proj matmul accumulates the residual via an
     extra identity-weight matmul against a host-provided bf16 copy of
     x; PSUM evicted on ACT with the proj bias folded in; DMA out. Both
     stages run 1-2 pairs behind the compute so waits never block the
     ACT/PE FIFOs.

Host-side preprocessing: weight transposes + bf16 casts + V-column
permute to [d*8+g] (AV multiply reads both operands unit-stride so DVE
2x mode applies) + bf16 x copy (residual matmul operand) + group
indicator matrices for the norm matmuls.

_cap_sync_waits: this walrus build accepts only ONE sync wait per compute
instruction; Tile emits more. The pass hoists excess waits onto
same-engine InstNoOps inserted immediately before the offender.
"""

import contextlib
import os

import numpy as np
import ml_dtypes

import concourse.bass as bass
import concourse.mybir as mybir
import concourse.tile as tile
from concourse.bass_utils import run_bass_kernel_spmd

F32 = mybir.dt.float32
BF16 = mybir.dt.bfloat16

B, C, HH, WW = 16, 512, 64, 64
N = HH * WW            # 4096
NB = 2                 # batches per core
NCORES = 8
NH, HD = 8, 64         # heads, head dim
GROUPS = 32
GSIZE = C // GROUPS    # 16 channels per group
EPS = 1e-5
CT = C // 128          # 4 channel tiles
NBLK = N // 128        # 32 position blocks per batch

AX = mybir.AxisListType
ALU = mybir.AluOpType
ACTF = mybir.ActivationFunctionType


def _bc(t, dims, extra_offset=0):
    """AP over tile/AP `t` with explicit free [step,count] dims (elem units)."""
    return bass.AP(tensor=t.tensor, offset=t.offset + extra_offset,
                   ap=[list(t.ap[0])] + [list(d) for d in dims])


def _cap_sync_waits(nc):
    """Walrus allows at most 1 sync wait per compute instruction; Tile can
    emit more. Hoist the excess onto same-engine InstNoOps inserted
    immediately before the offender."""
    import bass_rust
    n = 0
    for f in nc.m.functions:
        for blk in f.blocks:
            il = blk.instructions
            i = 0
            while i < len(il):
                ins = il[i]
                si = getattr(ins, "sync_info", None)
                if si is not None and si.on_wait and len(si.on_wait) > 1:
                    waits = list(si.on_wait)
                    for w in waits[:-1]:
                        nop = mybir.InstNoOp(name=f"W-abs-{n}", ins=[], outs=[])
                        n += 1
                        nop.engine = ins.engine
                        nop.sync_info = bass_rust.SyncInfo(on_wait=[w],
                                                           on_update=[])
                        il.insert(i, nop)
                        i += 1
                    si.on_wait = waits[-1:]
                i += 1
    return n


KDBG = int(os.environ.get("KDBG", "0"))


def build_kernel(nb=NB, nblk=NBLK, qk_bias=False, gp_u2_frac=0.0,
                 gp_t2=False, gp_t1_frac=0.0, gp_u1_frac=1.0,
                 period_ms=0.0, pq_bufs=2, pmm_bufs=1, qkv_bufs=3,
                 spool_bufs=2, gp_t2_frac=None, gp_stats=False,
                 gp_al=False, gp_t1l1_frac=0.0, tail_t2=0, tail_u2=0,
                 u1_dve_sub_every=1):
    n = nblk * 128
    npair = max(1, nblk // 2)      # block pairs (256 positions each)
    cs = min(512, n)               # proj/residual n-chunk
    nc = bass.Bass()

    x_d = nc.dram_tensor("x", [nb, C, n], F32, kind="ExternalInput")
    xbf_d = nc.dram_tensor("xbf", [nb, C, n], BF16, kind="ExternalInput")
    wqkvT_d = nc.dram_tensor("wqkvT", [C, 3 * C], BF16, kind="ExternalInput")
    pwT_d = nc.dram_tensor("pwT", [C, C], BF16, kind="ExternalInput")
    normw_d = nc.dram_tensor("normw", [C], F32, kind="ExternalInput")
    normb_d = nc.dram_tensor("normb", [C], F32, kind="ExternalInput")
    qkvb_d = nc.dram_tensor("qkvb", [3 * C], F32, kind="ExternalInput")
    pbeff_d = nc.dram_tensor("pbeff", [C], F32, kind="ExternalInput")
    ident_d = nc.dram_tensor("ident", [128, 128], BF16, kind="ExternalInput")
    ind_d = nc.dram_tensor("ind", [128, 8], BF16, kind="ExternalInput")
    indT_d = nc.dram_tensor("indT", [8, 128], BF16, kind="ExternalInput")
    out_d = nc.dram_tensor("out", [nb, C, n], F32, kind="ExternalOutput")

    with tile.TileContext(nc) as tc:
        def tt(eng, out, in0, in1, op):
            # (gpsimd scalar_tensor_tensor is priced better by the cost model
            # but does not compile for the Pool engine -- plain TT only)
            eng.tensor_tensor(out=out, in0=in0, in1=in1, op=op)

        def at(slot):
            """Virtual-clock stamp (scheduling-only; TimelineSim/HW replay is
            semaphore-timed). Shapes each engine's static FIFO order so ops
            that wait long never sit ahead of ops that are ready."""
            if period_ms <= 0:
                return contextlib.nullcontext()
            return tc.tile_wait_until(max(0.0, slot) * period_ms)

        with (
            tc.tile_pool(name="consts", bufs=1) as consts,
            tc.tile_pool(name="xpool", bufs=4) as xpool,
            tc.tile_pool(name="hlo", bufs=2) as hlo,
            tc.tile_pool(name="hhi", bufs=1) as hhi,
            tc.tile_pool(name="xbfp", bufs=2) as xbfp,
            tc.tile_pool(name="otr", bufs=2) as otr,
            tc.tile_pool(name="stats", bufs=2) as stats,
            tc.tile_pool(name="scb", bufs=2) as scb,
            tc.tile_pool(name="qkvsb", bufs=qkv_bufs) as qkvsb,
            tc.tile_pool(name="upool", bufs=4) as upool,
            tc.tile_pool(name="spool", bufs=spool_bufs) as spool,
            tc.tile_pool(name="outsb", bufs=3) as outsb,
            tc.tile_pool(name="pqkv", bufs=pq_bufs, space="PSUM") as pqkv,
            tc.tile_pool(name="pmm", bufs=pmm_bufs, space="PSUM") as pmm,
        ):
            # ---- constants / weights in SBUF ----
            wqkvT = []
            for c in range(CT):
                t = consts.tile([128, 3 * C], BF16, tag=f"wq{c}")
                nc.sync.dma_start(out=t, in_=wqkvT_d[c * 128:(c + 1) * 128, :])
                wqkvT.append(t)
            pwT = []
            for o in range(CT):
                t = consts.tile([128, C], BF16, tag=f"pw{o}")
                nc.sync.dma_start(out=t, in_=pwT_d[o * 128:(o + 1) * 128, :])
                pwT.append(t)
            ident = consts.tile([128, 128], BF16, tag="ident")
            nc.sync.dma_start(out=ident, in_=ident_d[:, :])
            ind = consts.tile([128, 8], BF16, tag="ind")
            nc.sync.dma_start(out=ind, in_=ind_d[:, :])
            indT = consts.tile([8, 128], BF16, tag="indT")
            nc.sync.dma_start(out=indT, in_=indT_d[:, :])
            nwt, nbt, pbt = [], [], []
            for c in range(CT):
                sl = slice(c * 128, (c + 1) * 128)
                t1 = consts.tile([128, 1], F32, tag=f"nw{c}")
                nc.sync.dma_start(out=t1, in_=normw_d[sl].rearrange("(p u) -> p u", u=1))
                nwt.append(t1)
                t2 = consts.tile([128, 1], F32, tag=f"nb{c}")
                nc.sync.dma_start(out=t2, in_=normb_d[sl].rearrange("(p u) -> p u", u=1))
                nbt.append(t2)
                t3 = consts.tile([128, 1], F32, tag=f"pb{c}")
                nc.sync.dma_start(out=t3, in_=pbeff_d[sl].rearrange("(p u) -> p u", u=1))
                pbt.append(t3)
            epst = consts.tile([8, 1], F32, tag="eps")
            nc.vector.memset(epst, 256.0 * EPS)
            qkbias = None
            if qk_bias:
                qkbias = consts.tile([128, 3 * C], F32, tag="qkb")
                nc.sync.dma_start(
                    out=qkbias,
                    in_=bass.AP(tensor=qkvb_d.ap().tensor, offset=0,
                                ap=[[0, 128], [1, 3 * C]]))

            def emit_norm(b, g0):
                """GroupNorm scales/biases + normalized h for batch b.

                x is streamed twice in [128, n/2] chunks (stats pass, then
                apply pass) so it never needs full SBUF residency."""
                nh2 = n // 2
                nsub = max(1, n // 512)
                # --- stats pass: stream x chunks, bn_stats into st[c] ---
                st_c = []
                sd = nc.vector.BN_STATS_DIM
                for c in range(CT):
                    st = stats.tile([128, nsub, sd], F32, tag=f"bnst{c}",
                                    name=f"bnst{c}")
                    st_c.append(st)
                for c in range(CT):
                    for half in range(2):
                        hs = slice(half * nh2, (half + 1) * nh2)
                        with at(g0 - 8 + (c * 2 + half) * 0.5):
                            t = xpool.tile([128, nh2], F32, tag="xs", name="xs")
                            nc.sync.dma_start(
                                out=t, in_=x_d[b, c * 128:(c + 1) * 128, hs])
                            xv = t.rearrange("p (s f) -> p s f", s=nsub // 2)
                            seng = nc.gpsimd if gp_stats else nc.vector
                            for s in range(nsub // 2):
                                seng.bn_stats(
                                    out=st_c[c][:, half * (nsub // 2) + s, :],
                                    in_=xv[:, s, :])
                # per-channel running stats -> st2all [128, (ct,2)] bf16
                ctx_norm = at(g0 - 4)
                ctx_norm.__enter__()
                st2all = stats.tile([128, 2 * CT], BF16, tag="st2all",
                                    name="st2all")
                for c in range(CT):
                    mv = stats.tile([128, nc.vector.BN_AGGR_DIM], F32, tag="bnmv")
                    nc.vector.bn_aggr(out=mv, in_=st_c[c])
                    # col0 = mean, col1 = E[x^2] = var + mean^2
                    nc.vector.tensor_copy(out=st2all[:, 2 * c:2 * c + 1],
                                          in_=mv[:, 0:1])
                    nc.vector.scalar_tensor_tensor(
                        out=st2all[:, 2 * c + 1:2 * c + 2], in0=mv[:, 0:1],
                        scalar=mv[:, 0:1], in1=mv[:, 1:2],
                        op0=ALU.mult, op1=ALU.add)
                if KDBG == 2:
                    sc_t, bi_t = [], []
                    for c in range(CT):
                        sc = scb.tile([128, 1], F32, tag=f"sc{c}", name=f"sc{c}")
                        bi = scb.tile([128, 1], F32, tag=f"bi{c}", name=f"bi{c}")
                        nc.vector.memset(sc, 1.0)
                        nc.vector.memset(bi, 0.0)
                        sc_t.append(sc)
                        bi_t.append(bi)
                    ctx_norm.__exit__(None, None, None)
                    ht = [[], []]
                    for half in range(2):
                        hp = hlo if half == 0 else hhi
                        hs = slice(half * nh2, (half + 1) * nh2)
                        for c in range(CT):
                            with at(g0 - 2.5 + (half * CT + c) * 0.25):
                                xa = xpool.tile([128, nh2], F32, tag="xs",
                                                name="xa")
                                nc.sync.dma_start(
                                    out=xa,
                                    in_=x_d[b, c * 128:(c + 1) * 128, hs])
                                t = hp.tile([128, nh2], BF16,
                                            tag=f"h{half}_{c}",
                                            name=f"h{half}_{c}")
                                nc.scalar.activation(out=t, in_=xa,
                                                     func=ACTF.Identity,
                                                     bias=bi_t[c],
                                                     scale=sc_t[c])
                            ht[half].append(t)
                    return ht
                # group aggregation: psum [8, (ct,2)] = sum over 16 channels
                # (full-size py-shaped tile so the PSUM tag stays uniform)
                pg_t = pmm.tile([128, 512], F32, tag="py", name="pg")
                pg = pg_t[0:8, 0:2 * CT]
                nc.tensor.matmul(pg, ind, st2all, start=True, stop=True)
                s8 = stats.tile([8, 2 * CT], F32, tag="s8", name="s8")
                nc.vector.tensor_copy(out=s8, in_=pg)
                # group math on 8 partitions, batched over ct via strided APs
                sm = _bc(s8, [(2, CT)])                  # sum of means
                se = _bc(s8, [(2, CT)], extra_offset=1)  # sum of E[x^2]
                m2 = stats.tile([8, CT], F32, tag="m2", name="m2")
                nc.vector.tensor_mul(m2, sm, sm)
                v256 = stats.tile([8, CT], F32, tag="v256", name="v256")
                nc.vector.scalar_tensor_tensor(
                    out=v256, in0=se, scalar=16.0, in1=m2,
                    op0=ALU.mult, op1=ALU.subtract)
                # rstd/16 = 1/sqrt(256(var+eps))
                sg = stats.tile([8, CT], F32, tag="sg", name="sg")
                nc.scalar.activation(out=sg, in_=v256, func=ACTF.Sqrt,
                                     scale=1.0, bias=epst)
                rg = stats.tile([8, CT], F32, tag="rg", name="rg")
                nc.vector.reciprocal(out=rg, in_=sg)
                # broadcast payload [8, (ct,2)] bf16: col0=rstd, col1=mean
                b8 = stats.tile([8, 2 * CT], BF16, tag="b8", name="b8")
                nc.vector.tensor_scalar(
                    out=_bc(b8, [(2, CT)]), in0=rg, scalar1=16.0,
                    scalar2=None, op0=ALU.mult)
                nc.vector.tensor_scalar(
                    out=_bc(b8, [(2, CT)], extra_offset=1), in0=sm,
                    scalar1=1.0 / 16.0, scalar2=None, op0=ALU.mult)
                pb_t = pmm.tile([128, 512], F32, tag="py", name="pb128")
                pb128 = pb_t[:, 0:2 * CT]
                nc.tensor.matmul(pb128, indT, b8, start=True, stop=True)
                c2 = scb.tile([128, 2 * CT], F32, tag="c2", name="c2")
                nc.vector.tensor_copy(out=c2, in_=pb128)
                # sc[ct] = rstd * nw ; bi[ct] = nb - mean*sc
                sc_t, bi_t = [], []
                for c in range(CT):
                    sc = scb.tile([128, 1], F32, tag=f"sc{c}", name=f"sc{c}")
                    bi = scb.tile([128, 1], F32, tag=f"bi{c}", name=f"bi{c}")
                    nc.vector.tensor_mul(sc, c2[:, 2 * c:2 * c + 1], nwt[c])
                    tmp = stats.tile([128, 1], F32, tag="tmp")
                    nc.vector.tensor_mul(tmp, c2[:, 2 * c + 1:2 * c + 2], sc)
                    nc.vector.tensor_sub(bi, nbt[c], tmp)
                    sc_t.append(sc)
                    bi_t.append(bi)
                ctx_norm.__exit__(None, None, None)
                # --- apply pass: re-stream x chunks -> normalized h ---
                ht = [[], []]
                for half in range(2):
                    hp = hlo if half == 0 else hhi
                    hs = slice(half * nh2, (half + 1) * nh2)
                    for c in range(CT):
                        with at(g0 - 2.5 + (half * CT + c) * 0.25):
                            xa = xpool.tile([128, nh2], F32, tag="xs", name="xa")
                            nc.sync.dma_start(
                                out=xa, in_=x_d[b, c * 128:(c + 1) * 128, hs])
                            t = hp.tile([128, nh2], BF16, tag=f"h{half}_{c}",
                                        name=f"h{half}_{c}")
                            nc.scalar.activation(out=t, in_=xa,
                                                 func=ACTF.Identity,
                                                 bias=bi_t[c], scale=sc_t[c])
                        ht[half].append(t)
                return ht

            ht_next = None
            for b in range(nb):
                if ht_next is None:
                    ht_next = emit_norm(b, b * npair)
                ht = ht_next
                ht_next = None
                bpc = cs // 128                 # blocks per out-chunk (4)
                otc = None
                pending = None
                pending_av = None
                pending_t2 = None
                pending_out = None
                def emit_qkv(pr):
                    g = b * npair + pr
                    blk0 = 2 * pr
                    qkv = qkvsb.tile([128, 2, 3 * C], BF16, tag="qkv")
                    nh2 = n // 2
                    hb = max(1, nblk // 2)
                    for sub in range(2):
                        blk = blk0 + sub
                        half = min(blk // hb, 1)
                        hslice = slice(blk * 128 - half * nh2,
                                       (blk + 1) * 128 - half * nh2)
                        with at(g - 0.7 + sub * 0.1):
                            p = pqkv.tile([128, 3 * C], F32, tag="pq",
                                          name=f"pq{sub}")
                            for c in range(CT):
                                lhsT = ht[half][c][:, hslice]
                                for oc in range(3):
                                    nc.tensor.matmul(
                                        p[:, oc * 512:(oc + 1) * 512], lhsT,
                                        wqkvT[c][:, oc * 512:(oc + 1) * 512],
                                        start=(c == 0), stop=(c == CT - 1))
                            if qkbias is not None:
                                nc.vector.tensor_add(out=qkv[:, sub, :], in0=p,
                                                     in1=qkbias)
                            else:
                                nc.scalar.copy(out=qkv[:, sub, :], in_=p)
                    return qkv

                qkv_next = emit_qkv(0)
                for pr in range(npair):
                    if pr == npair // 2 and b + 1 < nb:
                        # hoist next batch's GroupNorm into this batch's
                        # midsection so its stats/apply overlap attention
                        ht_next = emit_norm(b + 1, (b + 1) * npair)
                    g = b * npair + pr          # global pair slot
                    blk0 = 2 * pr               # first block of the pair
                    qkv = qkv_next

                    # q/k/v APs: qkv [128, (blk, 3C)]
                    QS = 3 * C
                    # ---- logits: u1[(blk,h,g,d)] = q[blk,h,d] * k[blk,g,d]
                    if KDBG == 1:
                        of = outsb.tile([128, 2 * C], F32, tag="out", name="ot")
                        nc.vector.tensor_copy(
                            out=of,
                            in_=_bc(qkv, [(QS, 2), (1, C)], extra_offset=1024))
                        nc.sync.dma_start(
                            out=out_d[b, 0:128, blk0 * 128:(blk0 + 2) * 128]
                                .rearrange("p (a f) -> p a f", a=2)
                                .rearrange("p a f -> p (a f)"),
                            in_=of[:, 0:256])
                        continue
                    u1 = upool.tile([128, 2 * NH * NH * HD], BF16, tag="u")
                    u1eng = nc.gpsimd if (pr % 10) < int(gp_u1_frac * 10 + 1e-6) \
                        else nc.vector
                    if pr == 0:
                        u1eng = nc.vector   # skip GP latency on the batch's first pair
                    ctx1 = at(g - 0.45); ctx1.__enter__()
                    # per-block 3D APs: the walrus ISA pattern caps compute
                    # APs at 3 free dims and the pair-strided 4D form does
                    # not merge
                    for sub in range(2):
                        eng_s = u1eng
                        if u1_dve_sub_every and sub == 1:
                            k = u1_dve_sub_every
                            hit = (pr % k == 0) if k > 0 else (pr % (-k) != 0)
                            if hit:
                                eng_s = nc.vector
                        tt(eng_s,
                           _bc(u1, [(NH * HD, NH), (HD, NH), (1, HD)],
                               extra_offset=sub * NH * NH * HD),
                           _bc(qkv, [(HD, NH), (0, NH), (1, HD)],
                               extra_offset=sub * QS),
                           _bc(qkv, [(0, NH), (HD, NH), (1, HD)],
                               extra_offset=sub * QS + 512),
                           ALU.mult)
                    ctx1.__exit__(None, None, None)
                    if pr + 1 < npair:
                        qkv_next = emit_qkv(pr + 1)
                    # d-tree: in-place halving on [p, (blk*64), d]
                    t1eng = nc.gpsimd if (pr % 10) < int(gp_t1_frac * 10 + 1e-6) \
                        else nc.vector
                    ctx2 = at(g + 0.05); ctx2.__enter__()
                    u1v = u1.rearrange("p (a d) -> p a d", d=HD)
                    w = HD
                    while w > 2:
                        eng = t1eng
                        if (w == HD and u1eng is nc.gpsimd
                                and (pr % 10) < int(gp_t1l1_frac * 10 + 1e-6)):
                            eng = nc.gpsimd   # L1 rides GP right after u1m
                        tt(eng, u1v[:, :, 0:w // 2], u1v[:, :, 0:w // 2],
                           u1v[:, :, w // 2:w], ALU.add)
                        w //= 2
                    s_l = spool.tile([128, 2 * NH * NH], BF16, tag="s")
                    nc.vector.tensor_tensor(
                        out=s_l.rearrange("p (a u) -> p a u", u=1),
                        in0=u1v[:, :, 0:1], in1=u1v[:, :, 1:2], op=ALU.add)
                    # softmax over g: E = exp(S/8); logits O(1) so no max-sub
                    # (high priority: DVE's d-sum stalls behind ACT's bulk
                    # copies otherwise -- strict per-engine FIFO)
                    ctx2.__exit__(None, None, None)
                    if KDBG == 5:
                        of = outsb.tile([128, 2 * C], F32, tag="out", name="ot")
                        nc.vector.tensor_copy(out=of[:, 0:128], in_=s_l)
                        nc.sync.dma_start(
                            out=out_d[b, 0:128, blk0 * 128:blk0 * 128 + 128],
                            in_=of[:, 0:128])
                        continue
                    e_l = spool.tile([128, 2 * NH * NH], BF16, tag="e")
                    with at(g + 0.38):
                        nc.scalar.activation(out=e_l, in_=s_l, func=ACTF.Exp,
                                             scale=0.125)
                    # deferred AV of the previous pair fills the exp-latency
                    # window on DVE (software pipeline, depth 2); tree2 of the
                    # pair before it is deferred one more slot
                    t2c = pending_av() if pending_av is not None else None
                    if pending_t2 is not None:
                        pending_t2()
                    pending_t2 = t2c
                    ctx3 = at(g + 0.42); ctx3.__enter__()
                    d_l = spool.tile([128, 2 * NH], F32, tag="d")
                    nc.vector.tensor_reduce(
                        out=d_l, in_=e_l.rearrange("p (h g) -> p h g", g=NH),
                        axis=AX.X, op=ALU.add)
                    r_l = spool.tile([128, 2 * NH], F32, tag="r")
                    nc.vector.reciprocal(out=r_l, in_=d_l)
                    a_l = spool.tile([128, 2 * NH * NH], BF16, tag="a")
                    aleng = nc.gpsimd if gp_al else nc.vector
                    tt(aleng, a_l.rearrange("p (h g) -> p h g", g=NH),
                       e_l.rearrange("p (h g) -> p h g", g=NH),
                       _bc(r_l, [(1, 2 * NH), (0, NH)]),
                       ALU.mult)
                    # ---- AV: u2[(blk,h,d,g)] = A[blk,h,g] * V'[blk,d,g]
                    # (V columns host-permuted to [d*8+g]: unit-stride reads)
                    ctx3.__exit__(None, None, None)

                    if KDBG == 3:
                        of = outsb.tile([128, 2 * C], F32, tag="out", name="ot")
                        nc.vector.tensor_copy(out=of[:, 0:128],
                                              in_=_bc(a_l, [(1, 128)]))
                        nc.sync.dma_start(
                            out=out_d[b, 0:128, blk0 * 128:blk0 * 128 + 128],
                            in_=of[:, 0:128])
                        continue

                    def make_av(a_l, qkv, blk0, pr, g):
                        def av():
                            ctx4 = at(g + 1.06)
                            ctx4.__enter__()
                            u2 = upool.tile([128, 2 * NH * HD * NH], BF16,
                                            tag="u")
                            u2eng = nc.gpsimd \
                                if (pr % 10) < int(gp_u2_frac * 10 + 1e-6) \
                                or (b == nb - 1 and pr >= npair - tail_u2) \
                                else nc.vector
                            for sub in range(2):
                                tt(u2eng,
                                   _bc(u2, [(HD * NH, NH), (NH, HD), (1, NH)],
                                       extra_offset=sub * NH * HD * NH),
                                   _bc(a_l, [(NH, NH), (0, HD), (1, NH)],
                                       extra_offset=sub * NH * NH),
                                   _bc(qkv, [(0, NH), (NH, HD), (1, NH)],
                                       extra_offset=sub * QS + 1024),
                                   ALU.mult)
                            ctx4.__exit__(None, None, None)

                            def t2():
                                nonlocal pending, otc
                                ctx5 = at(g + 2.06)
                                ctx5.__enter__()
                                # g-tree level 1 only (8 -> 4 partials); the
                                # remaining g-sum happens for free on the PE:
                                # the O-transposes accumulate the 4 partials
                                # in PSUM (transpose == matmul by identity)
                                f2 = gp_t2_frac if gp_t2_frac is not None \
                                    else (1.0 if gp_t2 else 0.0)
                                t2eng = nc.gpsimd \
                                    if (pr % 10) < int(f2 * 10 + 1e-6) \
                                    or (b == nb - 1 and pr >= npair - tail_t2) \
                                    else nc.vector
                                uv = u2.rearrange("p (a g) -> p a g", g=NH)
                                tt(t2eng, uv[:, :, 0:4], uv[:, :, 0:4],
                                   uv[:, :, 4:8], ALU.add)
                                ctx5.__exit__(None, None, None)
                                if pending is not None:
                                    pending()
                                pending = make_stage2(u2, blk0, pr, g)
                            return t2
                        return av
                    # ---- stage 2 (transpose + evict + proj), deferred one
                    # pair so ACT/PE FIFOs aren't blocked by waits on the
                    # GPSIMD tree of the current pair
                    def make_stage2(o_l, blk0, pr, g):
                        def stage2():
                            nonlocal otc
                            if blk0 % bpc == 0:
                                otc = otr.tile([128, CT * cs], BF16, tag="otc",
                                               name="otc")
                            for sub in range(2):
                                blk = blk0 + sub
                                with at(g + 2.02 + sub * 0.08):
                                    pt = pmm.tile([128, 512], F32, tag="pt")
                                    for ob in range(CT):
                                        for j in range(4):
                                            lhsT = _bc(
                                                o_l, [(NH, 128)],
                                                extra_offset=sub * NH * HD * NH
                                                + ob * 128 * NH + j)
                                            nc.tensor.matmul(
                                                pt[:, ob * 128:(ob + 1) * 128],
                                                lhsT, ident,
                                                start=(j == 0), stop=(j == 3))
                                    nc.scalar.copy(
                                        out=_bc(otc, [(cs, CT), (1, 128)],
                                                extra_offset=(blk % bpc) * 128),
                                        in_=_bc(pt, [(128, CT), (1, 128)]))
                            if (blk0 + 2) % bpc == 0:
                                nonlocal pending_out
                                j = blk0 // bpc
                                ncs = slice(j * cs, (j + 1) * cs)
                                pys = []
                                for c in range(CT):
                                    with at(g + 2.55 + c * 0.06):
                                        xr = xbfp.tile([128, cs], BF16, tag="xr",
                                                       name="xr")
                                        nc.sync.dma_start(
                                            out=xr,
                                            in_=xbf_d[b, c * 128:(c + 1) * 128,
                                                      ncs])
                                        py = pmm.tile([128, cs], F32, tag="py",
                                                      name="py")
                                        for ob in range(CT):
                                            nc.tensor.matmul(
                                                py,
                                                pwT[ob][:, c * 128:(c + 1) * 128],
                                                otc[:, ob * cs:(ob + 1) * cs],
                                                start=(ob == 0), stop=False)
                                        nc.tensor.matmul(py, ident, xr,
                                                         start=False, stop=True)
                                    pys.append(py)

                                def make_out(pys, ncs, g):
                                    def out_flush():
                                        for c in range(CT):
                                            with at(g + 3.35 + c * 0.06):
                                                ot = outsb.tile(
                                                    [128, cs], F32,
                                                    tag="out", name="ot")
                                                nc.scalar.activation(
                                                    out=ot, in_=pys[c],
                                                    func=ACTF.Identity,
                                                    bias=pbt[c], scale=1.0)
                                                nc.sync.dma_start(
                                                    out=out_d[b,
                                                              c * 128:(c + 1) * 128,
                                                              ncs],
                                                    in_=ot)
                                    return out_flush

                                if pending_out is not None:
                                    pending_out()
                                pending_out = make_out(pys, ncs, g)
                        return stage2

                    pending_av = make_av(a_l, qkv, blk0, pr, g)
                t2c = pending_av() if pending_av is not None else None
                pending_av = None
                if pending_t2 is not None:
                    pending_t2()
                if t2c is not None:
                    t2c()
                pending_t2 = None
                if pending is not None:
                    pending()
                    pending = None
                if pending_out is not None:
                    pending_out()
                    pending_out = None
    return nc


_CACHE = {}


def host_inputs(x, norm_w, norm_b, qkv_w, qkv_b, proj_w, proj_b):
    """Host-side preprocessing -> the kernel's shared input tensors."""
    bf = ml_dtypes.bfloat16
    # V-part column permutation: store V as [d*8+g] so the AV multiply reads
    # both operands at unit stride (DVE 2x mode).
    vperm = np.arange(3 * C)
    g_i, d_i = np.meshgrid(np.arange(NH), np.arange(HD), indexing="ij")
    vperm[2 * C:] = 2 * C + (d_i * NH + g_i).reshape(-1)   # old[g*64+d] -> new
    inv = np.empty_like(vperm)
    inv[vperm] = np.arange(3 * C)
    wq_p = qkv_w[inv]        # new column j holds old channel inv[j]
    qkvb_p = np.ascontiguousarray(qkv_b[inv])
    wqkvT = np.ascontiguousarray(wq_p.T).astype(bf)           # [C, 3C]
    pwT = np.ascontiguousarray(proj_w.T).astype(bf)           # [C(o), C(c)]
    ident = np.eye(128, dtype=np.float32).astype(bf)
    # group indicator: ind[c, g] = 1 if channel c (tile-local) in group g
    ind = np.zeros((128, 8), dtype=np.float32)
    for c in range(128):
        ind[c, c // GSIZE] = 1.0
    indT = np.ascontiguousarray(ind.T)
    return dict(wqkvT=wqkvT, pwT=pwT,
                normw=np.asarray(norm_w, np.float32),
                normb=np.asarray(norm_b, np.float32),
                qkvb=qkvb_p, pbeff=np.asarray(proj_b, np.float32),
                ident=ident, ind=ind.astype(bf), indT=indT.astype(bf))


def kernel(x, norm_w, norm_b, qkv_w, qkv_b, proj_w, proj_b):
    x = np.asarray(x, np.float32)
    norm_w = np.asarray(norm_w, np.float32)
    norm_b = np.asarray(norm_b, np.float32)
    qkv_w = np.asarray(qkv_w, np.float32)
    qkv_b = np.asarray(qkv_b, np.float32)
    proj_w = np.asarray(proj_w, np.float32)
    proj_b = np.asarray(proj_b, np.float32)

    qk_bias = bool(np.any(qkv_b != 0))
    key = ("full", qk_bias)
    if key not in _CACHE:
        nc_new = build_kernel(qk_bias=qk_bias)
        _cap_sync_waits(nc_new)   # HW path only; CoreSim rejects bare NoOps
        _CACHE[key] = nc_new
    nc = _CACHE[key]

    shared = host_inputs(x, norm_w, norm_b, qkv_w, qkv_b, proj_w, proj_b)
    xs = x.reshape(B, C, N)
    xbf = xs.astype(ml_dtypes.bfloat16)
    in_maps = [dict(x=np.ascontiguousarray(xs[c * NB:(c + 1) * NB]),
                    xbf=np.ascontiguousarray(xbf[c * NB:(c + 1) * NB]),
                    **shared)
               for c in range(NCORES)]
    res = run_bass_kernel_spmd(nc, in_maps, core_ids=list(range(NCORES)),
                               trace=bool(os.environ.get("KERNEL_TRACE")))
    global LAST_RES
    LAST_RES = res
    out = np.concatenate([res.results[c]["out"] for c in range(NCORES)], axis=0)
    return out.reshape(B, C, HH, WW).astype(np.float32)


LAST_RES = None
